# revision 1
# baseline (speedup 1.0000x reference)
"""Trainium2 Bass kernel for nn_MixedAttentionModule (CvT-style mixed attention block).

Data-parallel over batch: 32 batches -> 8 cores x 4 batches. No collectives.
All layouts channel-major on device (activations [C, n]); host pre-transposes x
and post-transposes the output. LN/BN/bias affines are folded into adjacent
weights on the host. Depthwise 3x3 convs run on the tensor engine as 9
diagonal matmuls accumulating in PSUM. Attention computes scores^T = k q^T so
the softmax denominator is a ones-matmul and attn@v needs no transpose.
"""
import sys

sys.path.insert(0, "/opt/trn_rl_repo")

import numpy as np
import ml_dtypes

B, n, C, NH, HD, FF = 32, 1024, 768, 12, 64, 3072
Ht = Wt = 32
M = 256          # kv positions (16*16)
NCORES = 8
BL = B // NCORES  # batches per core
EPS = 1e-5
KT = C // 128     # 6 channel tiles
FT = FF // 128    # 24 ff tiles
NCH = 2           # n-chunks of 512
F32 = None
BF16 = None

_BUILD_CACHE = {}


def _patch_compiler(ldw_opt=True):
    """Patch bass' walrus invocation: keep the standard pass list but allow
    toggling the LDWEIGHTS-dedup codegen optimization."""
    from pathlib import Path
    from concourse import bass_utils

    def patched(tmpdir, inp="bir.json", outp="file.neff", arch=None, *, dve_root=None):
        cmd = [
            bass_utils.get_walrus_driver(),
            "--pass",
            "birverifier,runtime_memory_reservation,lower_act,lower_dve,"
            "lower_ap_offset,codegen,neff_packager",
            "-i", inp,
            "--neff-output-filename", outp,
            "--enable-birsim=true",
            "--mem-mode=physical",
            "--policy=0",
            f"--enable-ldw-opt={'true' if ldw_opt else 'false'}",
            "--assign-static-dmas-to-sp=false",
            f"--dram-page-size={bass_utils.aot_getenv('NEURON_SCRATCHPAD_PAGE_SIZE', '256')}",
            "--enable-neff-debug-info=true",
            "--jobs", "8",
            *bass_utils.get_walrus_args(
                bass_utils.get_bir_arch(tmpdir, inp) if arch is None else arch,
                tmpdir, dve_root=dve_root,
            ),
        ]
        result = bass_utils.run_command(cmd, cwd=tmpdir)
        if result is not None:
            (Path(tmpdir) / "log.txt").write_text(result.stdout)
        return f"{tmpdir}/{outp}"

    bass_utils.bir_verify_and_optimise = patched


def _split_sync_waits(nc, max_waits=1):
    """walrus codegen in this environment allows at most one sync wait per
    instruction. Hoist excess waits onto standalone EventSemaphore carriers
    inserted just before, on the same engine (engines execute their stream
    in order, so this is equivalent)."""
    from concourse import mybir

    n_new = 0
    for f in nc.m.functions:
        for blk in f.blocks:
            out = []
            for inst in blk.instructions:
                si = getattr(inst, "sync_info", None)
                if si is not None:
                    waits = list(si.on_wait or [])
                    ups = list(si.on_update or [])
                    if len(waits) > max_waits:
                        extra = waits[: len(waits) - max_waits]
                        keep = waits[len(waits) - max_waits:]
                        for w in extra:
                            n_new += 1
                            out.append(mybir.InstEventSemaphore(
                                name=f"syncw-{n_new}-{inst.name}",
                                ins=[], outs=[],
                                engine=inst.engine,
                                sync_info=mybir.SyncInfo(on_wait=[w], on_update=[]),
                            ))
                        inst.sync_info = mybir.SyncInfo(on_wait=keep, on_update=ups)
                out.append(inst)
            blk.instructions = out
    return n_new


def _build_program():
    from concourse import bass, mybir, tile

    f32 = mybir.dt.float32
    bf16 = mybir.dt.bfloat16
    Alu = mybir.AluOpType
    Act = mybir.ActivationFunctionType

    nc = bass.Bass("TRN2", target_bir_lowering=False, debug=False, num_devices=NCORES)

    # ---- DRAM I/O ----
    xTf = nc.dram_tensor("xTf", [BL, C, n], f32, kind="ExternalInput").ap()
    xTb = nc.dram_tensor("xTb", [BL, C, n], bf16, kind="ExternalInput").ap()
    wqT = nc.dram_tensor("wqT", [C, C], bf16, kind="ExternalInput").ap()
    wkT = nc.dram_tensor("wkT", [C, C], bf16, kind="ExternalInput").ap()
    wvT = nc.dram_tensor("wvT", [C, C], bf16, kind="ExternalInput").ap()
    w1p = nc.dram_tensor("w1p", [FT, 128, C], bf16, kind="ExternalInput").ap()
    w2T = nc.dram_tensor("w2T", [FF, C], bf16, kind="ExternalInput").ap()
    dq9 = nc.dram_tensor("dq9", [KT, 128, 9 * 128], bf16, kind="ExternalInput").ap()
    dk9 = nc.dram_tensor("dk9", [KT, 128, 9 * 128], bf16, kind="ExternalInput").ap()
    dv9 = nc.dram_tensor("dv9", [KT, 128, 9 * 128], bf16, kind="ExternalInput").ap()
    bq_d = nc.dram_tensor("bq", [C, 1], f32, kind="ExternalInput").ap()
    bk_d = nc.dram_tensor("bk", [C, 1], f32, kind="ExternalInput").ap()
    bva_d = nc.dram_tensor("bva", [C, 1], f32, kind="ExternalInput").ap()
    b1_d = nc.dram_tensor("b1", [FF, 1], f32, kind="ExternalInput").ap()
    ones_k_d = nc.dram_tensor("ones_k", [128, 1], bf16, kind="ExternalInput").ap()
    ones_r_d = nc.dram_tensor("ones_r", [1, 128], bf16, kind="ExternalInput").ap()
    ones_sq_d = nc.dram_tensor("ones_sq", [128, 128], bf16, kind="ExternalInput").ap()
    outT = nc.dram_tensor("outT", [BL, C, n], f32, kind="ExternalOutput").ap()

    with tile.TileContext(nc) as tc:
        with tc.tile_pool(name="P", bufs=1) as P:
            # ---- persistent SBUF (weights + per-batch activations) ----
            wq_sb = [P.tile([128, C], bf16, name=f"wq{k}", tag="wq", bufs=KT) for k in range(KT)]
            wk_sb = [P.tile([128, C], bf16, name=f"wk{k}", tag="wk", bufs=KT) for k in range(KT)]
            wv_sb = [P.tile([128, C], bf16, name=f"wv{k}", tag="wv", bufs=KT) for k in range(KT)]
            bq_sb = [P.tile([128, 1], f32, name=f"bq{k}", tag="bq", bufs=KT) for k in range(KT)]
            bk_sb = [P.tile([128, 1], f32, name=f"bk{k}", tag="bk", bufs=KT) for k in range(KT)]
            bva_sb = [P.tile([128, 1], f32, name=f"bva{k}", tag="bva", bufs=KT) for k in range(KT)]
            b1_sb = [P.tile([128, 1], f32, name=f"b1_{t}", tag="b1", bufs=FT) for t in range(FT)]
            ones_k = P.tile([128, 1], bf16, name="onesk", tag="onesk", bufs=1)
            ones_r = P.tile([1, 128], bf16, name="onesr", tag="onesr", bufs=1)
            ones_sq = P.tile([128, 128], bf16, name="onessq", tag="onessq", bufs=1)

            for k in range(KT):
                nc.sync.dma_start(wq_sb[k][:], wqT[k * 128:(k + 1) * 128, :])
                nc.sync.dma_start(wk_sb[k][:], wkT[k * 128:(k + 1) * 128, :])
                nc.sync.dma_start(wv_sb[k][:], wvT[k * 128:(k + 1) * 128, :])
                nc.sync.dma_start(bq_sb[k][:], bq_d[k * 128:(k + 1) * 128, :])
                nc.sync.dma_start(bk_sb[k][:], bk_d[k * 128:(k + 1) * 128, :])
                nc.sync.dma_start(bva_sb[k][:], bva_d[k * 128:(k + 1) * 128, :])
            for t in range(FT):
                nc.sync.dma_start(b1_sb[t][:], b1_d[t * 128:(t + 1) * 128, :])
            nc.sync.dma_start(ones_k[:], ones_k_d[:, :])
            nc.sync.dma_start(ones_r[:], ones_r_d[:, :])
            nc.sync.dma_start(ones_sq[:], ones_sq_d[:, :])

            pad = [P.tile([128, 34, 34], bf16, name=f"pad{k}", tag="pad", bufs=KT) for k in range(KT)]
            xb_sb = [P.tile([128, n], bf16, name=f"xb{k}", tag="xb", bufs=KT) for k in range(KT)]
            actq = [P.tile([128, n], bf16, name=f"aq{k}", tag="aq", bufs=KT) for k in range(KT)]
            actk = [P.tile([128, M], bf16, name=f"ak{k}", tag="ak", bufs=KT) for k in range(KT)]
            actv = [P.tile([128, M], bf16, name=f"av{k}", tag="av", bufs=KT) for k in range(KT)]
            qT = [P.tile([128, n], bf16, name=f"qT{k}", tag="qT", bufs=KT) for k in range(KT)]
            kTt = [P.tile([128, M], bf16, name=f"kT{k}", tag="kT", bufs=KT) for k in range(KT)]
            vtok = [P.tile([128, C], bf16, name=f"vt{k}", tag="vt", bufs=2) for k in range(2)]
            OT = [P.tile([128, n], bf16, name=f"OT{k}", tag="OT", bufs=KT) for k in range(KT)]
            x2b = [P.tile([128, n], bf16, name=f"x2{k}", tag="x2", bufs=KT) for k in range(KT)]
            xl2 = [P.tile([128, n], bf16, name=f"xl2{k}", tag="xl2", bufs=KT) for k in range(KT)]

            # zero the padded conv buffers once (interiors are overwritten per batch;
            # the one-element borders must stay zero)
            for k in range(KT):
                nc.vector.memset(pad[k][:], 0.0)

            # constant APs for float biases of activation ops
            czero = P.tile([128, 1], f32, name="czero", tag="cz", bufs=2)
            nc.vector.memset(czero[:], 0.0)
            nc.const_aps.aps[(f32, 0.0)] = czero[:]
            ceps = P.tile([128, 1], f32, name="ceps", tag="cz", bufs=2)
            nc.vector.memset(ceps[:], EPS)
            nc.const_aps.aps[(f32, EPS)] = ceps[:]

            def ln_stats_apply(src_tiles, dst_write, label, bno):
                """src_tiles: 6 bf16 [128, n] channel-major tiles.
                Stats matmuls use a ones[128,128] stationary so the channel-sums
                arrive pre-broadcast across all 128 partitions; all row math is
                then full-width DVE work and no PE broadcast is needed."""
                with tc.tile_pool(name=f"ps_ln_{label}{bno}", bufs=1, space="PSUM") as psp:
                    ps_mean = [psp.tile([128, 512], f32, name=f"psm{label}{bno}_{c}", tag="mm", bufs=4) for c in range(NCH)]
                    ps_msq = [psp.tile([128, 512], f32, name=f"psq{label}{bno}_{c}", tag="mm", bufs=4) for c in range(NCH)]
                    # squares on ACT (bf16 out), then ones-matmul stats; groups are
                    # interleaved across banks so sq tiles can double-buffer
                    for k in range(KT):
                        sqt = P.tile([128, n], bf16, name=f"sq{label}{bno}_{k}", tag="sq", bufs=2)
                        nc.scalar.activation(sqt[:], src_tiles[k][:], Act.Square)
                        for ch in range(NCH):
                            sl = slice(ch * 512, (ch + 1) * 512)
                            nc.tensor.matmul(ps_mean[ch][:], ones_sq[:], src_tiles[k][:, sl],
                                             start=(k == 0), stop=(k == KT - 1))
                            nc.tensor.matmul(ps_msq[ch][:], ones_sq[:], sqt[:, sl],
                                             start=(k == 0), stop=(k == KT - 1))
                    for ch in range(NCH):
                        sl = slice(ch * 512, (ch + 1) * 512)
                        mbc = P.tile([128, 512], f32, name=f"mbc{label}{bno}_{ch}", tag="mbc", bufs=2)
                        rbc = P.tile([128, 512], f32, name=f"rbc{label}{bno}_{ch}", tag="rbc", bufs=2)
                        nc.vector.tensor_scalar_mul(mbc[:], ps_mean[ch][:], 1.0 / C)
                        # rstd = 1/sqrt((msq/C) - mean^2 + eps)
                        nc.vector.tensor_mul(rbc[:], mbc[:], mbc[:])
                        nc.vector.scalar_tensor_tensor(rbc[:], ps_msq[ch][:], 1.0 / C,
                                                       rbc[:], Alu.mult, Alu.subtract)
                        # rstd = exp(-0.5*ln(var+eps)) on ACT (keeps DVE free;
                        # table accuracy ~1e-4 rel, far below bf16 noise)
                        nc.scalar.activation(rbc[:], rbc[:], Act.Ln, bias=EPS)
                        nc.scalar.activation(rbc[:], rbc[:], Act.Exp, scale=-0.5)
                        for k in range(KT):
                            dst_write(k, ch, src_tiles[k][:, sl], mbc, rbc)

            for b in range(BL):
                # ---------------- LN1 + conv + projections ----------------
                for k in range(KT):
                    nc.sync.dma_start(xb_sb[k][:], xTb[b, k * 128:(k + 1) * 128, :])

                def ln1_write(k, ch, src, mbc, rbc):
                    tmp = P.tile([128, 512], f32, name=f"t1_{b}_{k}_{ch}", tag="tmp", bufs=3)
                    nc.vector.tensor_sub(tmp[:], src, mbc[:])
                    # write normalized values into padded interior rows
                    r0 = 1 + 16 * ch
                    dst = pad[k][:, r0:r0 + 16, 1:33]
                    nc.vector.tensor_mul(dst, tmp[:].rearrange("p (a c) -> p a c", a=16), rbc[:].rearrange("p (a c) -> p a c", a=16))
                    return

                ln_stats_apply(xb_sb, ln1_write, "a", b)

                # conv (9 diagonal matmuls per output chunk) + exact ELU
                with tc.tile_pool(name=f"ps_conv{b}", bufs=1, space="PSUM") as cvp:
                    def elu_chain(ps_ap, dst_ap, width):
                        tmin = P.tile([128, width], f32, name=f"tm{b}", tag="tmpe", bufs=3)
                        et = P.tile([128, width], bf16, name=f"ee{b}", tag="ee", bufs=3)
                        nc.vector.tensor_scalar_min(tmin[:], ps_ap, 0.0)
                        nc.scalar.activation(et[:], tmin[:], Act.Exp)
                        # elu+1 = relu(x) + exp(min(x,0));  the -1 is folded into proj biases
                        nc.vector.scalar_tensor_tensor(dst_ap, ps_ap, 0.0, et[:], Alu.max, Alu.add)

                    for k in range(KT):
                        dqt = P.tile([128, 9 * 128], bf16, name=f"dq{b}_{k}", tag="dq", bufs=2)
                        nc.gpsimd.dma_start(dqt[:], dq9[k, :, :])
                        pq = [cvp.tile([128, 512], f32, name=f"pcq{b}_{k}_{c}", tag="mm", bufs=4) for c in range(NCH)]
                        for tap in range(9):
                            dy, dx = tap // 3, tap % 3
                            for ch in range(NCH):
                                rhs = pad[k][:, dy + 16 * ch:dy + 16 * ch + 16, dx:dx + 32]
                                nc.tensor.matmul(pq[ch][:], dqt[:, tap * 128:(tap + 1) * 128], rhs,
                                                 start=(tap == 0), stop=(tap == 8))
                        for ch in range(NCH):
                            elu_chain(pq[ch][:], actq[k][:, ch * 512:(ch + 1) * 512], 512)
                    for k in range(KT):
                        dkt = P.tile([128, 9 * 128], bf16, name=f"dk{b}_{k}", tag="dkv", bufs=2)
                        dvt = P.tile([128, 9 * 128], bf16, name=f"dv{b}_{k}", tag="dkv", bufs=2)
                        nc.gpsimd.dma_start(dkt[:], dk9[k, :, :])
                        nc.gpsimd.dma_start(dvt[:], dv9[k, :, :])
                        pk = cvp.tile([128, M], f32, name=f"pck{b}_{k}", tag="mm", bufs=4)
                        pv = cvp.tile([128, M], f32, name=f"pcv{b}_{k}", tag="mm", bufs=4)
                        for tap in range(9):
                            dy, dx = tap // 3, tap % 3
                            rhs = pad[k][:, dy:dy + 32:2, dx:dx + 32:2]
                            nc.tensor.matmul(pk[:], dkt[:, tap * 128:(tap + 1) * 128], rhs,
                                             start=(tap == 0), stop=(tap == 8))
                            nc.tensor.matmul(pv[:], dvt[:, tap * 128:(tap + 1) * 128], rhs,
                                             start=(tap == 0), stop=(tap == 8))
                        elu_chain(pk[:], actk[k][:, :], M)
                        elu_chain(pv[:], actv[k][:, :], M)

                    # projections -- both n-chunks per stationary weight so
                    # consecutive matmuls reuse the loaded weights
                    for mt in range(KT):
                        psq = [cvp.tile([128, 512], f32, name=f"pq{b}_{mt}_{c}", tag="mm", bufs=4)
                               for c in range(NCH)]
                        for k in range(KT):
                            for ch in range(NCH):
                                nc.tensor.matmul(psq[ch][:], wq_sb[k][:, mt * 128:(mt + 1) * 128],
                                                 actq[k][:, ch * 512:(ch + 1) * 512],
                                                 start=(k == 0), stop=(k == KT - 1))
                        for ch in range(NCH):
                            nc.scalar.activation(qT[mt][:, ch * 512:(ch + 1) * 512], psq[ch][:],
                                                 Act.Identity, bias=bq_sb[mt][:])
                    for mt in range(KT):
                        ps = cvp.tile([128, M], f32, name=f"pk{b}_{mt}", tag="mm", bufs=4)
                        for k in range(KT):
                            nc.tensor.matmul(ps[:], wk_sb[k][:, mt * 128:(mt + 1) * 128], actk[k][:, :],
                                             start=(k == 0), stop=(k == KT - 1))
                        nc.scalar.activation(kTt[mt][:, :], ps[:], Act.Identity, bias=bk_sb[mt][:])
                    for mt2 in range(2):
                        psv = [cvp.tile([128, w], f32, name=f"pv{b}_{mt2}_{c}", tag="mm", bufs=4)
                               for c, w in [(0, 512), (1, 256)]]
                        for k in range(KT):
                            for ch, w in [(0, 512), (1, 256)]:
                                nc.tensor.matmul(psv[ch][:], actv[k][:, mt2 * 128:(mt2 + 1) * 128],
                                                 wv_sb[k][:, ch * 512:ch * 512 + w],
                                                 start=(k == 0), stop=(k == KT - 1))
                        for ch, w in [(0, 512), (1, 256)]:
                            nc.vector.tensor_copy(vtok[mt2][:, ch * 512:ch * 512 + w], psv[ch][:])

                # ---------------- attention ----------------
                with tc.tile_pool(name=f"ps_at{b}", bufs=1, space="PSUM") as atp:
                    for j in range(NH // 2):
                        ET_h = {}
                        # per-head inverse sums packed into the pair's partition halves
                        sinv = [P.tile([128, 512], f32, name=f"si{b}_{j}_{c}", tag="sinv", bufs=4)
                                for c in range(NCH)]
                        for hh in range(2):
                            bp = 64 * hh
                            ET_h[hh] = [P.tile([128, n], bf16, name=f"ET{b}_{j}_{hh}_{mt}", tag="ET", bufs=4)
                                        for mt in range(2)]
                            for mt in range(2):
                                for ch in range(NCH):
                                    ps = atp.tile([128, 512], f32, name=f"pss{b}_{j}_{hh}_{mt}_{ch}", tag="mm", bufs=4)
                                    nc.tensor.matmul(ps[:],
                                                     kTt[j][bp:bp + 64, mt * 128:(mt + 1) * 128],
                                                     qT[j][bp:bp + 64, ch * 512:(ch + 1) * 512],
                                                     tile_position=(bp, 0))
                                    nc.scalar.activation(ET_h[hh][mt][:, ch * 512:(ch + 1) * 512], ps[:],
                                                         Act.Exp, scale=0.125)
                            # colsum via ones[128,128]: result rows are the sum
                            # broadcast across all partitions
                            for ch in range(NCH):
                                sum_ps = atp.tile([128, 512], f32, name=f"psum{b}_{j}_{hh}_{ch}", tag="bc", bufs=2)
                                for mt in range(2):
                                    nc.tensor.matmul(sum_ps[:], ones_sq[:],
                                                     ET_h[hh][mt][:, ch * 512:(ch + 1) * 512],
                                                     start=(mt == 0), stop=(mt == 1))
                                # 1/s = exp(-ln(s)) on ACT
                                nc.scalar.activation(sinv[ch][bp:bp + 64, :],
                                                     sum_ps[bp:bp + 64, :], Act.Ln)
                                nc.scalar.activation(sinv[ch][bp:bp + 64, :],
                                                     sinv[ch][bp:bp + 64, :], Act.Exp, scale=-1.0)
                        po = [atp.tile([128, 512], f32, name=f"po{b}_{j}_{c}", tag="o", bufs=2)
                              for c in range(NCH)]
                        for hh in range(2):
                            bp = 64 * hh
                            h = 2 * j + hh
                            for mt in range(2):
                                for ch in range(NCH):
                                    nc.tensor.matmul(po[ch][bp:bp + 64, :],
                                                     vtok[mt][:, h * 64:(h + 1) * 64],
                                                     ET_h[hh][mt][:, ch * 512:(ch + 1) * 512],
                                                     start=(mt == 0), stop=(mt == 1),
                                                     tile_position=(0, bp))
                        for ch in range(NCH):
                            sl = slice(ch * 512, (ch + 1) * 512)
                            nc.vector.tensor_mul(OT[j][:, sl], po[ch][:], sinv[ch][:])

                # ---------------- residual + LN2 ----------------
                for k in range(KT):
                    for ch in range(NCH):
                        sl = slice(ch * 512, (ch + 1) * 512)
                        xf = P.tile([128, 512], f32, name=f"xf{b}_{k}_{ch}", tag="xf", bufs=3)
                        nc.sync.dma_start(xf[:], xTf[b, k * 128:(k + 1) * 128, sl])
                        nc.vector.scalar_tensor_tensor(x2b[k][:, sl], OT[k][:, sl], bva_sb[k][:], xf[:],
                                                       Alu.add, Alu.add)

                def ln2_write(k, ch, src, mbc, rbc):
                    tmp = P.tile([128, 512], f32, name=f"t2_{b}_{k}_{ch}", tag="tmp", bufs=3)
                    nc.vector.tensor_sub(tmp[:], src, mbc[:])
                    nc.vector.tensor_mul(xl2[k][:, ch * 512:(ch + 1) * 512], tmp[:], rbc[:])

                ln_stats_apply(x2b, ln2_write, "c", b)

                # ---------------- FFN + final residual ----------------
                with tc.tile_pool(name=f"ps_ffn{b}", bufs=1, space="PSUM") as ffp:
                    for ch in range(NCH):
                        sl = slice(ch * 512, (ch + 1) * 512)
                        ph2 = [ffp.tile([128, 512], f32, name=f"ph2_{b}_{ch}_{mt}", tag="h2", bufs=6)
                               for mt in range(KT)]
                        for ft in range(FT):
                            w1b = P.tile([128, C], bf16, name=f"w1_{b}_{ch}_{ft}", tag="w1", bufs=3)
                            nc.gpsimd.dma_start(w1b[:], w1p[ft, :, :])
                            w2b = P.tile([128, C], bf16, name=f"w2_{b}_{ch}_{ft}", tag="w2", bufs=3)
                            nc.gpsimd.dma_start(w2b[:], w2T[ft * 128:(ft + 1) * 128, :])
                            ph1 = ffp.tile([128, 512], f32, name=f"ph1_{b}_{ch}_{ft}", tag="h1", bufs=2)
                            for k in range(KT):
                                nc.tensor.matmul(ph1[:], w1b[:, k * 128:(k + 1) * 128], xl2[k][:, sl],
                                                 start=(k == 0), stop=(k == KT - 1))
                            gt = P.tile([128, 512], bf16, name=f"g_{b}_{ch}_{ft}", tag="g", bufs=3)
                            nc.scalar.activation(gt[:], ph1[:], Act.Gelu, bias=b1_sb[ft][:])
                            for mt in range(KT):
                                nc.tensor.matmul(ph2[mt][:], w2b[:, mt * 128:(mt + 1) * 128], gt[:],
                                                 start=(ft == 0), stop=(ft == FT - 1))
                        for mt in range(KT):
                            xf2 = P.tile([128, 512], f32, name=f"xf2_{b}_{ch}_{mt}", tag="xf", bufs=3)
                            nc.sync.dma_start(xf2[:], xTf[b, mt * 128:(mt + 1) * 128, sl])
                            ub = P.tile([128, 512], f32, name=f"u_{b}_{ch}_{mt}", tag="tmp", bufs=3)
                            nc.vector.scalar_tensor_tensor(ub[:], OT[mt][:, sl], bva_sb[mt][:], xf2[:],
                                                           Alu.add, Alu.add)
                            ob = P.tile([128, 512], f32, name=f"o_{b}_{ch}_{mt}", tag="ob", bufs=3)
                            nc.vector.tensor_add(ob[:], ub[:], ph2[mt][:])
                            nc.sync.dma_start(outT[b, mt * 128:(mt + 1) * 128, sl], ob[:])
    n_hoisted = _split_sync_waits(nc)
    print(f"_split_sync_waits: hoisted waits onto {n_hoisted} carrier instructions")
    return nc


def _host_prep(inputs):
    """Fold LN/BN affines into weights; build packed bf16 arrays."""
    f = lambda k: np.asarray(inputs[k], np.float32)
    bfc = lambda a: np.ascontiguousarray(a.astype(ml_dtypes.bfloat16))
    x = f("x")                         # (B, n, C)
    ln1_g, ln1_b = f("ln1_g"), f("ln1_b")
    ln2_g, ln2_b = f("ln2_g"), f("ln2_b")

    prep = {}
    xT = np.ascontiguousarray(x.transpose(0, 2, 1))   # (B, C, n)
    prep["xTf"] = xT
    prep["xTb"] = bfc(xT)

    diag9 = {}
    badj = {}
    for nm in ["q", "k", "v"]:
        w = f(f"dw_w_{nm}")[:, 0]                     # (C,3,3)
        w_eff = w * ln1_g[:, None, None]
        cb = f(f"dw_b_{nm}") + ln1_b * w.sum((1, 2))  # exact only if ln1_b == 0 (boundary)
        assert np.abs(cb).max() < 1e-30, "nonzero conv bias not implemented on device"
        sc = f(f"bn_g_{nm}") / np.sqrt(f(f"bn_v_{nm}") + EPS)
        sh = f(f"bn_b_{nm}") - f(f"bn_m_{nm}") * sc
        W = f(f"W_{nm}")
        W_eff = W * sc[None, :]
        # device computes elu+1 (the -1 is folded here); also BN shift
        b_eff = f(f"b_{nm}") + W @ sh - W_eff.sum(1)
        # pack 9 taps of diagonal matrices: [KT, 128, 9*128]
        d = np.zeros((KT, 128, 9 * 128), np.float32)
        for kt in range(KT):
            ww = w_eff[kt * 128:(kt + 1) * 128]       # (128,3,3)
            for tap in range(9):
                dy, dx = tap // 3, tap % 3
                d[kt, np.arange(128), tap * 128 + np.arange(128)] = ww[:, dy, dx]
        diag9[nm] = bfc(d)
        badj[nm] = b_eff
        prep[f"w{nm}T"] = bfc(np.ascontiguousarray(W_eff.T))
    prep["dq9"], prep["dk9"], prep["dv9"] = diag9["q"], diag9["k"], diag9["v"]
    prep["bq"] = badj["q"].reshape(C, 1)
    prep["bk"] = badj["k"].reshape(C, 1)
    prep["bva"] = badj["v"].reshape(C, 1)

    W1 = f("W1") * ln2_g[None, :]                     # (FF, C)
    b1 = f("b1") + f("W1") @ ln2_b
    W2 = f("W2")                                      # (C, FF)
    assert np.abs(f("b2")).max() < 1e-30, "nonzero b2 not implemented on device"
    W1T = W1.T                                        # (C, FF) = [cin, f]
    w1p = np.zeros((FT, 128, C), np.float32)          # [ft, cin_p, kt*128+f]
    for ft in range(FT):
        blk = W1T[:, ft * 128:(ft + 1) * 128]         # (C, 128)
        w1p[ft] = blk.reshape(KT, 128, 128).transpose(1, 0, 2).reshape(128, C)
    prep["w1p"] = bfc(w1p)
    prep["w2T"] = bfc(np.ascontiguousarray(W2.T))     # (FF, C)
    prep["b1"] = b1.reshape(FF, 1)
    prep["ones_k"] = np.ones((128, 1), ml_dtypes.bfloat16)
    prep["ones_r"] = np.ones((1, 128), ml_dtypes.bfloat16)
    prep["ones_sq"] = np.ones((128, 128), ml_dtypes.bfloat16)
    return prep


def kernel(**inputs):
    from concourse.bass_utils import run_bass_kernel_spmd

    _patch_compiler(ldw_opt=_BUILD_CACHE.get("ldw_opt", False))
    if "nc" not in _BUILD_CACHE:
        _BUILD_CACHE["nc"] = _build_program()
    nc = _BUILD_CACHE["nc"]

    prep = _host_prep(inputs)
    shared = {k: v for k, v in prep.items() if not k.startswith("xT")}
    in_maps = []
    for c in range(NCORES):
        im = dict(shared)
        im["xTf"] = np.ascontiguousarray(prep["xTf"][c * BL:(c + 1) * BL])
        im["xTb"] = np.ascontiguousarray(prep["xTb"][c * BL:(c + 1) * BL])
        in_maps.append(im)

    res = run_bass_kernel_spmd(nc, in_maps, list(range(NCORES)),
                               **_BUILD_CACHE.get("run_kwargs", {}))
    _BUILD_CACHE["last_results"] = res
    outs = [res.results[c]["outT"].transpose(0, 2, 1) for c in range(NCORES)]
    return np.ascontiguousarray(np.concatenate(outs, 0).astype(np.float32))



# revision 29
# speedup vs baseline: 1.1920x; 1.1920x over previous
"""Trainium2 Bass kernel for nn_MixedAttentionModule (CvT-style mixed attention block).

Data-parallel over batch: 32 batches -> 8 cores x 4 batches. No collectives.
All layouts channel-major on device (activations [C, n]); host pre-transposes x
and post-transposes the output. LN/BN/bias affines are folded into adjacent
weights on the host. Depthwise 3x3 convs run on the tensor engine as 9
diagonal matmuls accumulating in PSUM. Attention computes scores^T = k q^T so
the softmax denominator is a ones-matmul and attn@v needs no transpose.
"""
import sys

sys.path.insert(0, "/opt/trn_rl_repo")

import numpy as np
import ml_dtypes

B, n, C, NH, HD, FF = 32, 1024, 768, 12, 64, 3072
Ht = Wt = 32
M = 256          # kv positions (16*16)
NCORES = 8
BL = B // NCORES  # batches per core
EPS = 1e-5
KT = C // 128     # 6 channel tiles
FT = FF // 128    # 24 ff tiles
NCH = 2           # n-chunks of 512
SX = 8.0          # fp8 scale on LN2 output (|ln| <= sqrt(C)=27.7, *8 = 222 < 240)
LN_SX = 2.0794415416798357   # ln(SX), folded into the rstd exp
SV = 16.0         # fp8 scale on v tokens (|v| ~ 0.8, *16 = 13 << 240)
NLN_SV = -2.772588722239781  # -ln(SV), folded into the sinv exp
F32 = None
BF16 = None

_BUILD_CACHE = {}


def _patch_compiler(ldw_opt=True):
    """Patch bass' walrus invocation: keep the standard pass list but allow
    toggling the LDWEIGHTS-dedup codegen optimization."""
    from pathlib import Path
    from concourse import bass_utils

    def patched(tmpdir, inp="bir.json", outp="file.neff", arch=None, *, dve_root=None):
        cmd = [
            bass_utils.get_walrus_driver(),
            "--pass",
            "birverifier,runtime_memory_reservation,lower_act,lower_dve,"
            "lower_ap_offset,codegen,neff_packager",
            "-i", inp,
            "--neff-output-filename", outp,
            "--enable-birsim=true",
            "--mem-mode=physical",
            "--policy=0",
            f"--enable-ldw-opt={'true' if ldw_opt else 'false'}",
            "--assign-static-dmas-to-sp=false",
            f"--dram-page-size={bass_utils.aot_getenv('NEURON_SCRATCHPAD_PAGE_SIZE', '256')}",
            "--enable-neff-debug-info=true",
            "--jobs", "8",
            *bass_utils.get_walrus_args(
                bass_utils.get_bir_arch(tmpdir, inp) if arch is None else arch,
                tmpdir, dve_root=dve_root,
            ),
        ]
        result = bass_utils.run_command(cmd, cwd=tmpdir)
        if result is not None:
            (Path(tmpdir) / "log.txt").write_text(result.stdout)
        return f"{tmpdir}/{outp}"

    bass_utils.bir_verify_and_optimise = patched


def _split_sync_waits(nc, max_waits=1):
    """walrus codegen in this environment allows at most one sync wait per
    instruction. Hoist excess waits onto standalone EventSemaphore carriers
    inserted just before, on the same engine (engines execute their stream
    in order, so this is equivalent)."""
    from concourse import mybir

    n_new = 0
    for f in nc.m.functions:
        for blk in f.blocks:
            out = []
            for inst in blk.instructions:
                si = getattr(inst, "sync_info", None)
                if si is not None:
                    waits = list(si.on_wait or [])
                    ups = list(si.on_update or [])
                    if len(waits) > max_waits:
                        extra = waits[: len(waits) - max_waits]
                        keep = waits[len(waits) - max_waits:]
                        for w in extra:
                            n_new += 1
                            out.append(mybir.InstEventSemaphore(
                                name=f"syncw-{n_new}-{inst.name}",
                                ins=[], outs=[],
                                engine=inst.engine,
                                sync_info=mybir.SyncInfo(on_wait=[w], on_update=[]),
                            ))
                        inst.sync_info = mybir.SyncInfo(on_wait=keep, on_update=ups)
                out.append(inst)
            blk.instructions = out
    return n_new


def _build_program():
    from concourse import bass, mybir, tile

    f32 = mybir.dt.float32
    bf16 = mybir.dt.bfloat16
    Alu = mybir.AluOpType
    Act = mybir.ActivationFunctionType
    DRM = mybir.MatmulPerfMode.DoubleRow

    f8 = mybir.dt.float8e4

    nc = bass.Bass("TRN2", target_bir_lowering=False, debug=False, num_devices=NCORES)

    # ---- DRAM I/O ----
    xTf = nc.dram_tensor("xTf", [BL, C, n], f32, kind="ExternalInput").ap()
    xTb = nc.dram_tensor("xTb", [BL, C, n], bf16, kind="ExternalInput").ap()
    wqT = nc.dram_tensor("wqT", [C, C], bf16, kind="ExternalInput").ap()
    wkT = nc.dram_tensor("wkT", [C, C], bf16, kind="ExternalInput").ap()
    wvT = nc.dram_tensor("wvT", [C, C], bf16, kind="ExternalInput").ap()
    # fp8 W1 packed for DoubleRow; bf16 W2 packed per ft-tile:
    #   w1q[p, ft*768 + kp*256 + i*128 + f] = W1eff[ft*128+f, (2kp+i)*128+p] * s1
    #   w2r[p, ft*768 + mt*128 + m] = W2[mt*128+m, ft*128+p]
    w1q = nc.dram_tensor("w1q", [128, FT * C], f8, kind="ExternalInput").ap()
    w2r = nc.dram_tensor("w2r", [128, FT * C], bf16, kind="ExternalInput").ap()
    sc1_d = nc.dram_tensor("sc1", [128, 1], f32, kind="ExternalInput").ap()
    dq9 = nc.dram_tensor("dq9", [KT, 128, 9 * 128], bf16, kind="ExternalInput").ap()
    dk9 = nc.dram_tensor("dk9", [KT, 128, 9 * 128], bf16, kind="ExternalInput").ap()
    dv9 = nc.dram_tensor("dv9", [KT, 128, 9 * 128], bf16, kind="ExternalInput").ap()
    bq_d = nc.dram_tensor("bq", [C, 1], f32, kind="ExternalInput").ap()
    bk_d = nc.dram_tensor("bk", [C, 1], f32, kind="ExternalInput").ap()
    bva_d = nc.dram_tensor("bva", [C, 1], f32, kind="ExternalInput").ap()
    b1_d = nc.dram_tensor("b1", [FF, 1], f32, kind="ExternalInput").ap()
    ones_k_d = nc.dram_tensor("ones_k", [128, 1], bf16, kind="ExternalInput").ap()
    ones_r_d = nc.dram_tensor("ones_r", [1, 128], bf16, kind="ExternalInput").ap()
    ones_sq_d = nc.dram_tensor("ones_sq", [128, 128], bf16, kind="ExternalInput").ap()
    outT = nc.dram_tensor("outT", [BL, C, n], f32, kind="ExternalOutput").ap()

    with tile.TileContext(nc) as tc:
        with tc.tile_pool(name="P", bufs=1) as P:
            # ---- persistent SBUF (weights + per-batch activations) ----
            wq_sb = [P.tile([128, C], bf16, name=f"wq{k}", tag="wq", bufs=KT) for k in range(KT)]
            wk_sb = [P.tile([128, C], bf16, name=f"wk{k}", tag="wk", bufs=KT) for k in range(KT)]
            wv_sb = [P.tile([128, C], bf16, name=f"wv{k}", tag="wv", bufs=KT) for k in range(KT)]
            bq_sb = [P.tile([128, 1], f32, name=f"bq{k}", tag="bq", bufs=KT) for k in range(KT)]
            bk_sb = [P.tile([128, 1], f32, name=f"bk{k}", tag="bk", bufs=KT) for k in range(KT)]
            bva_sb = [P.tile([128, 1], f32, name=f"bva{k}", tag="bva", bufs=KT) for k in range(KT)]
            b1_sb = [P.tile([128, 1], f32, name=f"b1_{t}", tag="b1", bufs=FT) for t in range(FT)]
            ones_k = P.tile([128, 1], bf16, name="onesk", tag="onesk", bufs=1)
            ones_r = P.tile([1, 128], bf16, name="onesr", tag="onesr", bufs=1)
            ones_sq = P.tile([128, 128], bf16, name="onessq", tag="onessq", bufs=1)
            w1_sb = P.tile([128, FT * C], f8, name="w1q", tag="w1q", bufs=1)
            sc1_sb = P.tile([128, 1], f32, name="sc1", tag="sc1", bufs=1)

            # ones first (LN1 stats needs them immediately), then the small bias
            # vectors; the heavy projection weights go on the gpsimd queue so
            # batch 0's x tiles aren't stuck behind them on the sync queue.
            nc.sync.dma_start(ones_sq[:], ones_sq_d[:, :])
            nc.sync.dma_start(ones_k[:], ones_k_d[:, :])
            nc.sync.dma_start(ones_r[:], ones_r_d[:, :])
            for k in range(KT):
                nc.sync.dma_start(bq_sb[k][:], bq_d[k * 128:(k + 1) * 128, :])
                nc.sync.dma_start(bk_sb[k][:], bk_d[k * 128:(k + 1) * 128, :])
                nc.sync.dma_start(bva_sb[k][:], bva_d[k * 128:(k + 1) * 128, :])
            for t in range(FT):
                nc.sync.dma_start(b1_sb[t][:], b1_d[t * 128:(t + 1) * 128, :])
            nc.sync.dma_start(sc1_sb[:], sc1_d[:, :])
            for k in range(KT):
                nc.gpsimd.dma_start(wq_sb[k][:], wqT[k * 128:(k + 1) * 128, :])
                nc.gpsimd.dma_start(wk_sb[k][:], wkT[k * 128:(k + 1) * 128, :])
                nc.gpsimd.dma_start(wv_sb[k][:], wvT[k * 128:(k + 1) * 128, :])

            pad = [P.tile([128, 34, 34], bf16, name=f"pad{k}", tag="pad", bufs=KT) for k in range(KT)]
            xb_sb = [P.tile([128, n], bf16, name=f"xb{k}", tag="xb", bufs=KT) for k in range(KT)]
            actq = [P.tile([128, n], bf16, name=f"aq{k}", tag="aq", bufs=KT) for k in range(KT)]
            actk = [P.tile([128, M], bf16, name=f"ak{k}", tag="ak", bufs=KT) for k in range(KT)]
            actv = [P.tile([128, M], bf16, name=f"av{k}", tag="av", bufs=KT) for k in range(KT)]
            qT = [P.tile([128, n], bf16, name=f"qT{k}", tag="qT", bufs=KT) for k in range(KT)]
            kTt = [P.tile([128, M], bf16, name=f"kT{k}", tag="kT", bufs=KT) for k in range(KT)]
            vt8 = P.tile([128, 2, C], f8, name="vt8", tag="vt", bufs=1)
            ones8 = P.tile([128, 256], f8, name="ones8", tag="ones8", bufs=1)
            nc.vector.memset(ones8[:], 1.0)
            OT = [P.tile([128, n], bf16, name=f"OT{k}", tag="OT", bufs=KT) for k in range(KT)]
            x2b = [P.tile([128, n], bf16, name=f"x2{k}", tag="x2", bufs=KT) for k in range(KT)]
            # LN2 output: fp8, all 6 channel tiles in one buffer so DoubleRow can
            # pair adjacent k-tiles along the free dim (stride n between planes)
            xl8 = P.tile([128, KT * n], f8, name="xl8", tag="xl8", bufs=1)

            # zero the padded conv buffers once (interiors are overwritten per batch;
            # the one-element borders must stay zero)
            for k in range(KT):
                nc.vector.memset(pad[k][:], 0.0)

            # constant APs for float biases of activation ops
            czero = P.tile([128, 1], f32, name="czero", tag="cz", bufs=2)
            nc.vector.memset(czero[:], 0.0)
            nc.const_aps.aps[(f32, 0.0)] = czero[:]
            ceps = P.tile([128, 1], f32, name="ceps", tag="cz", bufs=2)
            nc.vector.memset(ceps[:], EPS)
            nc.const_aps.aps[(f32, EPS)] = ceps[:]
            cln8 = P.tile([128, 1], f32, name="cln8", tag="cln8", bufs=1)
            nc.vector.memset(cln8[:], LN_SX)
            nc.const_aps.aps[(f32, LN_SX)] = cln8[:]
            cnv = P.tile([128, 1], f32, name="cnv", tag="cnv", bufs=1)
            nc.vector.memset(cnv[:], NLN_SV)
            nc.const_aps.aps[(f32, NLN_SV)] = cnv[:]

            def ln_stats_apply(src_tiles, dst_write, label, bno, rstd_bias=0.0):
                """src_tiles: 6 bf16 [128, n] channel-major tiles.
                Stats matmuls use a ones[128,128] stationary so the channel-sums
                arrive pre-broadcast across all 128 partitions; all row math is
                then full-width DVE work and no PE broadcast is needed.
                rstd_bias: added inside the exp -> multiplies rstd by
                exp(rstd_bias) for free (fp8 pre-scaling)."""
                with tc.tile_pool(name=f"ps_ln_{label}{bno}", bufs=1, space="PSUM") as psp:
                    ps_mean = [psp.tile([128, 512], f32, name=f"psm{label}{bno}_{c}", tag="mm", bufs=4) for c in range(NCH)]
                    ps_msq = [psp.tile([128, 512], f32, name=f"psq{label}{bno}_{c}", tag="mm", bufs=4) for c in range(NCH)]
                    # squares on ACT (bf16 out), then ones-matmul stats; groups are
                    # interleaved across banks so sq tiles can double-buffer
                    for k in range(KT):
                        sqt = P.tile([128, n], bf16, name=f"sq{label}{bno}_{k}", tag="sq", bufs=2)
                        nc.scalar.activation(sqt[:], src_tiles[k][:], Act.Square)
                        for ch in range(NCH):
                            sl = slice(ch * 512, (ch + 1) * 512)
                            nc.tensor.matmul(ps_mean[ch][:], ones_sq[:], src_tiles[k][:, sl],
                                             start=(k == 0), stop=(k == KT - 1))
                            nc.tensor.matmul(ps_msq[ch][:], ones_sq[:], sqt[:, sl],
                                             start=(k == 0), stop=(k == KT - 1))
                    for ch in range(NCH):
                        sl = slice(ch * 512, (ch + 1) * 512)
                        mbc = P.tile([128, 512], f32, name=f"mbc{label}{bno}_{ch}", tag="mbc", bufs=2)
                        rbc = P.tile([128, 512], f32, name=f"rbc{label}{bno}_{ch}", tag="rbc", bufs=2)
                        nc.vector.tensor_scalar_mul(mbc[:], ps_mean[ch][:], 1.0 / C)
                        # rstd = 1/sqrt((msq/C) - mean^2 + eps)
                        nc.vector.tensor_mul(rbc[:], mbc[:], mbc[:])
                        nc.vector.scalar_tensor_tensor(rbc[:], ps_msq[ch][:], 1.0 / C,
                                                       rbc[:], Alu.mult, Alu.subtract)
                        # rstd = exp(-0.5*ln(var+eps)) on ACT (keeps DVE free;
                        # table accuracy ~1e-4 rel, far below bf16 noise)
                        nc.scalar.activation(rbc[:], rbc[:], Act.Ln, bias=EPS)
                        nc.scalar.activation(rbc[:], rbc[:], Act.Exp, scale=-0.5,
                                             bias=rstd_bias)
                        for k in range(KT):
                            dst_write(k, ch, src_tiles[k][:, sl], mbc, rbc)

            for b in range(BL):
                # ---------------- LN1 + conv + projections ----------------
                for k in range(KT):
                    nc.sync.dma_start(xb_sb[k][:], xTb[b, k * 128:(k + 1) * 128, :])

                def ln1_write(k, ch, src, mbc, rbc):
                    tmp = P.tile([128, 512], f32, name=f"t1_{b}_{k}_{ch}", tag="tmp", bufs=3)
                    nc.vector.tensor_sub(tmp[:], src, mbc[:])
                    # write normalized values into padded interior rows
                    r0 = 1 + 16 * ch
                    dst = pad[k][:, r0:r0 + 16, 1:33]
                    nc.vector.tensor_mul(dst, tmp[:].rearrange("p (a c) -> p a c", a=16), rbc[:].rearrange("p (a c) -> p a c", a=16))
                    return

                ln_stats_apply(xb_sb, ln1_write, "a", b)

                # conv (9 diagonal matmuls per output chunk) + exact ELU
                with tc.tile_pool(name=f"ps_conv{b}", bufs=1, space="PSUM") as cvp:
                    def elu_chain(ps_ap, dst_ap, width):
                        tmin = P.tile([128, width], f32, name=f"tm{b}", tag="tmpe", bufs=3)
                        et = P.tile([128, width], bf16, name=f"ee{b}", tag="ee", bufs=3)
                        nc.vector.tensor_scalar_min(tmin[:], ps_ap, 0.0)
                        nc.scalar.activation(et[:], tmin[:], Act.Exp)
                        # elu+1 = relu(x) + exp(min(x,0));  the -1 is folded into proj biases
                        nc.vector.scalar_tensor_tensor(dst_ap, ps_ap, 0.0, et[:], Alu.max, Alu.add)

                    for k in range(KT):
                        dqt = P.tile([128, 9 * 128], bf16, name=f"dq{b}_{k}", tag="dq", bufs=2)
                        nc.gpsimd.dma_start(dqt[:], dq9[k, :, :])
                        pq = [cvp.tile([128, 512], f32, name=f"pcq{b}_{k}_{c}", tag="mm", bufs=4) for c in range(NCH)]
                        for tap in range(9):
                            dy, dx = tap // 3, tap % 3
                            for ch in range(NCH):
                                rhs = pad[k][:, dy + 16 * ch:dy + 16 * ch + 16, dx:dx + 32]
                                nc.tensor.matmul(pq[ch][:], dqt[:, tap * 128:(tap + 1) * 128], rhs,
                                                 start=(tap == 0), stop=(tap == 8))
                        for ch in range(NCH):
                            elu_chain(pq[ch][:], actq[k][:, ch * 512:(ch + 1) * 512], 512)
                    for k in range(KT):
                        dkt = P.tile([128, 9 * 128], bf16, name=f"dk{b}_{k}", tag="dkv", bufs=2)
                        dvt = P.tile([128, 9 * 128], bf16, name=f"dv{b}_{k}", tag="dkv", bufs=2)
                        nc.gpsimd.dma_start(dkt[:], dk9[k, :, :])
                        nc.gpsimd.dma_start(dvt[:], dv9[k, :, :])
                        pk = cvp.tile([128, M], f32, name=f"pck{b}_{k}", tag="mm", bufs=4)
                        pv = cvp.tile([128, M], f32, name=f"pcv{b}_{k}", tag="mm", bufs=4)
                        for tap in range(9):
                            dy, dx = tap // 3, tap % 3
                            rhs = pad[k][:, dy:dy + 32:2, dx:dx + 32:2]
                            nc.tensor.matmul(pk[:], dkt[:, tap * 128:(tap + 1) * 128], rhs,
                                             start=(tap == 0), stop=(tap == 8))
                            nc.tensor.matmul(pv[:], dvt[:, tap * 128:(tap + 1) * 128], rhs,
                                             start=(tap == 0), stop=(tap == 8))
                        elu_chain(pk[:], actk[k][:, :], M)
                        elu_chain(pv[:], actv[k][:, :], M)

                    # projections -- both n-chunks per stationary weight so
                    # consecutive matmuls reuse the loaded weights
                    for mt in range(KT):
                        psq = [cvp.tile([128, 512], f32, name=f"pq{b}_{mt}_{c}", tag="mm", bufs=4)
                               for c in range(NCH)]
                        for k in range(KT):
                            for ch in range(NCH):
                                nc.tensor.matmul(psq[ch][:], wq_sb[k][:, mt * 128:(mt + 1) * 128],
                                                 actq[k][:, ch * 512:(ch + 1) * 512],
                                                 start=(k == 0), stop=(k == KT - 1))
                        for ch in range(NCH):
                            nc.scalar.activation(qT[mt][:, ch * 512:(ch + 1) * 512], psq[ch][:],
                                                 Act.Identity, bias=bq_sb[mt][:])
                    for mt in range(KT):
                        ps = cvp.tile([128, M], f32, name=f"pk{b}_{mt}", tag="mm", bufs=4)
                        for k in range(KT):
                            nc.tensor.matmul(ps[:], wk_sb[k][:, mt * 128:(mt + 1) * 128], actk[k][:, :],
                                             start=(k == 0), stop=(k == KT - 1))
                        nc.scalar.activation(kTt[mt][:, :], ps[:], Act.Identity, bias=bk_sb[mt][:])
                    for mt2 in range(2):
                        psv = [cvp.tile([128, w], f32, name=f"pv{b}_{mt2}_{c}", tag="mm", bufs=4)
                               for c, w in [(0, 512), (1, 256)]]
                        for k in range(KT):
                            for ch, w in [(0, 512), (1, 256)]:
                                nc.tensor.matmul(psv[ch][:], actv[k][:, mt2 * 128:(mt2 + 1) * 128],
                                                 wv_sb[k][:, ch * 512:ch * 512 + w],
                                                 start=(k == 0), stop=(k == KT - 1))
                        for ch, w in [(0, 512), (1, 256)]:
                            # v tokens in fp8, prescaled by SV (folded out via sinv)
                            nc.vector.tensor_scalar_mul(
                                vt8[:, mt2:mt2 + 1, ch * 512:ch * 512 + w], psv[ch][:], SV)

                if b == 0:
                    # one-time fp8 W1 load; queued here so batch 0's conv
                    # weights (same gpsimd queue) aren't delayed behind it
                    for half in range(4):
                        slh = slice(half * (FT * C // 4), (half + 1) * (FT * C // 4))
                        nc.gpsimd.dma_start(w1_sb[:, slh], w1q[:, slh])

                # ---------------- attention ----------------
                with tc.tile_pool(name=f"ps_at{b}", bufs=1, space="PSUM") as atp:
                    for j in range(NH // 2):
                        # per-head inverse sums packed into the pair's partition halves
                        sinv = [P.tile([128, 512], f32, name=f"si{b}_{j}_{c}", tag="sinv", bufs=4)
                                for c in range(NCH)]
                        # exp(scores) in fp8, kv tiles stacked for DoubleRow
                        ET2 = [P.tile([128, 2, n], f8, name=f"ET{b}_{j}_{hh}", tag="ET", bufs=3)
                               for hh in range(2)]
                        # scores: alternate the two heads' row-halves so the PE
                        # streams both halves concurrently
                        for mt in range(2):
                            for ch in range(NCH):
                                for hh in range(2):
                                    bp = 64 * hh
                                    ps = atp.tile([128, 512], f32, name=f"pss{b}_{j}_{hh}_{mt}_{ch}", tag="mm", bufs=4)
                                    nc.tensor.matmul(ps[:],
                                                     kTt[j][bp:bp + 64, mt * 128:(mt + 1) * 128],
                                                     qT[j][bp:bp + 64, ch * 512:(ch + 1) * 512],
                                                     tile_position=(bp, 0))
                                    nc.scalar.activation(ET2[hh][:, mt:mt + 1, ch * 512:(ch + 1) * 512],
                                                         ps[:], Act.Exp, scale=0.125)
                        # kv-sum: one fp8 DoubleRow ones-matmul per head/chunk,
                        # result broadcast across all partitions
                        for hh in range(2):
                            bp = 64 * hh
                            for ch in range(NCH):
                                sum_ps = atp.tile([128, 512], f32, name=f"psum{b}_{j}_{hh}_{ch}", tag="bc", bufs=2)
                                nc.tensor.matmul(sum_ps[:],
                                                 ones8[:].rearrange("p (i f) -> p i f", i=2),
                                                 ET2[hh][:, :, ch * 512:(ch + 1) * 512],
                                                 perf_mode=DRM)
                                # 1/(s*SV) = exp(-ln(s) - ln SV) on ACT
                                nc.scalar.activation(sinv[ch][bp:bp + 64, :],
                                                     sum_ps[bp:bp + 64, :], Act.Ln)
                                nc.scalar.activation(sinv[ch][bp:bp + 64, :],
                                                     sinv[ch][bp:bp + 64, :], Act.Exp, scale=-1.0,
                                                     bias=NLN_SV)
                        po = [atp.tile([128, 512], f32, name=f"po{b}_{j}_{c}", tag="o", bufs=2)
                              for c in range(NCH)]
                        for mt in range(2):
                            for ch in range(NCH):
                                for hh in range(2):
                                    bp = 64 * hh
                                    h = 2 * j + hh
                                    nc.tensor.matmul(po[ch][bp:bp + 64, :],
                                                     vt8[:, mt:mt + 1, h * 64:(h + 1) * 64],
                                                     ET2[hh][:, mt:mt + 1, ch * 512:(ch + 1) * 512],
                                                     start=(mt == 0), stop=(mt == 1),
                                                     tile_position=(0, bp))
                        for ch in range(NCH):
                            sl = slice(ch * 512, (ch + 1) * 512)
                            nc.vector.tensor_mul(OT[j][:, sl], po[ch][:], sinv[ch][:])

                # ---------------- residual + LN2 ----------------
                for k in range(KT):
                    for ch in range(NCH):
                        sl = slice(ch * 512, (ch + 1) * 512)
                        xf = P.tile([128, 512], f32, name=f"xf{b}_{k}_{ch}", tag="xf", bufs=3)
                        nc.sync.dma_start(xf[:], xTf[b, k * 128:(k + 1) * 128, sl])
                        nc.vector.scalar_tensor_tensor(x2b[k][:, sl], OT[k][:, sl], bva_sb[k][:], xf[:],
                                                       Alu.add, Alu.add)

                def ln2_write(k, ch, src, mbc, rbc):
                    tmp = P.tile([128, 512], f32, name=f"t2_{b}_{k}_{ch}", tag="tmp", bufs=3)
                    nc.vector.tensor_sub(tmp[:], src, mbc[:])
                    # rbc carries exp(ln 8) = SX, so this writes xn*8 in fp8e4
                    nc.vector.tensor_mul(xl8[:, k * n + ch * 512:k * n + (ch + 1) * 512],
                                         tmp[:], rbc[:])

                ln_stats_apply(x2b, ln2_write, "c", b, rstd_bias=LN_SX)

                # ---------------- FFN (fp8 DoubleRow h1, bf16 h2) + residual ----------------
                with tc.tile_pool(name=f"ps_ffn{b}", bufs=1, space="PSUM") as ffp:
                    for ch in range(NCH):
                        sl = slice(ch * 512, (ch + 1) * 512)
                        ph2 = [ffp.tile([128, 512], f32, name=f"ph2_{b}_{ch}_{mt}", tag="h2", bufs=6)
                               for mt in range(KT)]
                        for ft in range(FT):
                            w2b = P.tile([128, C], bf16, name=f"w2_{b}_{ch}_{ft}", tag="w2", bufs=3)
                            nc.gpsimd.dma_start(w2b[:], w2r[:, ft * C:(ft + 1) * C])
                            ph1 = ffp.tile([128, 512], f32, name=f"ph1_{b}_{ch}_{ft}", tag="h1", bufs=2)
                            for kp in range(KT // 2):
                                w1ap = w1_sb[:, ft * C + kp * 256: ft * C + (kp + 1) * 256] \
                                    .rearrange("p (i f) -> p i f", i=2)
                                xap = xl8[:, 2 * kp * n:(2 * kp + 2) * n] \
                                    .rearrange("p (i t) -> p i t", i=2)[:, :, sl]
                                nc.tensor.matmul(ph1[:], w1ap, xap,
                                                 start=(kp == 0), stop=(kp == KT // 2 - 1),
                                                 perf_mode=DRM)
                            gt = P.tile([128, 512], bf16, name=f"g_{b}_{ch}_{ft}", tag="g", bufs=3)
                            nc.scalar.activation(gt[:], ph1[:], Act.Gelu,
                                                 bias=b1_sb[ft][:], scale=sc1_sb[:])
                            for mt in range(KT):
                                nc.tensor.matmul(ph2[mt][:],
                                                 w2b[:, mt * 128:(mt + 1) * 128],
                                                 gt[:],
                                                 start=(ft == 0), stop=(ft == FT - 1))
                        for mt in range(KT):
                            xf2 = P.tile([128, 512], f32, name=f"xf2_{b}_{ch}_{mt}", tag="xf", bufs=3)
                            nc.sync.dma_start(xf2[:], xTf[b, mt * 128:(mt + 1) * 128, sl])
                            ub = P.tile([128, 512], f32, name=f"u_{b}_{ch}_{mt}", tag="tmp", bufs=3)
                            nc.vector.scalar_tensor_tensor(ub[:], OT[mt][:, sl], bva_sb[mt][:], xf2[:],
                                                           Alu.add, Alu.add)
                            ob = P.tile([128, 512], f32, name=f"o_{b}_{ch}_{mt}", tag="ob", bufs=3)
                            nc.vector.tensor_add(ob[:], ub[:], ph2[mt][:])
                            nc.sync.dma_start(outT[b, mt * 128:(mt + 1) * 128, sl], ob[:])
    n_hoisted = _split_sync_waits(nc)
    print(f"_split_sync_waits: hoisted waits onto {n_hoisted} carrier instructions")
    return nc


def _host_prep(inputs):
    """Fold LN/BN affines into weights; build packed bf16 arrays."""
    f = lambda k: np.asarray(inputs[k], np.float32)
    bfc = lambda a: np.ascontiguousarray(a.astype(ml_dtypes.bfloat16))
    x = f("x")                         # (B, n, C)
    ln1_g, ln1_b = f("ln1_g"), f("ln1_b")
    ln2_g, ln2_b = f("ln2_g"), f("ln2_b")

    prep = {}
    xT = np.ascontiguousarray(x.transpose(0, 2, 1))   # (B, C, n)
    prep["xTf"] = xT
    prep["xTb"] = bfc(xT)

    diag9 = {}
    badj = {}
    for nm in ["q", "k", "v"]:
        w = f(f"dw_w_{nm}")[:, 0]                     # (C,3,3)
        w_eff = w * ln1_g[:, None, None]
        cb = f(f"dw_b_{nm}") + ln1_b * w.sum((1, 2))  # exact only if ln1_b == 0 (boundary)
        assert np.abs(cb).max() < 1e-30, "nonzero conv bias not implemented on device"
        sc = f(f"bn_g_{nm}") / np.sqrt(f(f"bn_v_{nm}") + EPS)
        sh = f(f"bn_b_{nm}") - f(f"bn_m_{nm}") * sc
        W = f(f"W_{nm}")
        W_eff = W * sc[None, :]
        # device computes elu+1 (the -1 is folded here); also BN shift
        b_eff = f(f"b_{nm}") + W @ sh - W_eff.sum(1)
        # pack 9 taps of diagonal matrices: [KT, 128, 9*128]
        d = np.zeros((KT, 128, 9 * 128), np.float32)
        for kt in range(KT):
            ww = w_eff[kt * 128:(kt + 1) * 128]       # (128,3,3)
            for tap in range(9):
                dy, dx = tap // 3, tap % 3
                d[kt, np.arange(128), tap * 128 + np.arange(128)] = ww[:, dy, dx]
        diag9[nm] = bfc(d)
        badj[nm] = b_eff
        prep[f"w{nm}T"] = bfc(np.ascontiguousarray(W_eff.T))
    prep["dq9"], prep["dk9"], prep["dv9"] = diag9["q"], diag9["k"], diag9["v"]
    prep["bq"] = badj["q"].reshape(C, 1)
    prep["bk"] = badj["k"].reshape(C, 1)
    prep["bva"] = badj["v"].reshape(C, 1)

    W1 = f("W1") * ln2_g[None, :]                     # (FF, C)
    b1 = f("b1") + f("W1") @ ln2_b
    W2 = f("W2")                                      # (C, FF)
    assert np.abs(f("b2")).max() < 1e-30, "nonzero b2 not implemented on device"
    # fp8e4 (TRN: max +-240) DoubleRow packing, power-of-2 per-tensor scale
    f8c = lambda a: np.clip(a, -240.0, 240.0).astype(ml_dtypes.float8_e4m3)
    s1 = 2.0 ** np.floor(np.log2(224.0 / max(np.abs(W1).max(), 1e-30)))
    # w1q[p, ft*768 + kp*256 + i*128 + f] = W1[ft*128+f, (2kp+i)*128+p] * s1
    w1q = (W1 * s1).reshape(FT, 128, KT, 128).transpose(3, 0, 2, 1).reshape(128, FT * C)
    # w2r[p, ft*768 + mt*128 + m] = W2[mt*128+m, ft*128+p]
    w2r = W2.T.reshape(FT, 128, C).transpose(1, 0, 2).reshape(128, FT * C)
    prep["w1q"] = np.ascontiguousarray(f8c(w1q))
    prep["w2r"] = bfc(w2r)
    prep["sc1"] = np.full((128, 1), 1.0 / (s1 * SX), np.float32)
    prep["b1"] = b1.reshape(FF, 1)
    prep["ones_k"] = np.ones((128, 1), ml_dtypes.bfloat16)
    prep["ones_r"] = np.ones((1, 128), ml_dtypes.bfloat16)
    prep["ones_sq"] = np.ones((128, 128), ml_dtypes.bfloat16)
    return prep


def kernel(**inputs):
    from concourse.bass_utils import run_bass_kernel_spmd

    _patch_compiler(ldw_opt=_BUILD_CACHE.get("ldw_opt", False))
    if "nc" not in _BUILD_CACHE:
        _BUILD_CACHE["nc"] = _build_program()
    nc = _BUILD_CACHE["nc"]

    prep = _host_prep(inputs)
    shared = {k: v for k, v in prep.items() if not k.startswith("xT")}
    in_maps = []
    for c in range(NCORES):
        im = dict(shared)
        im["xTf"] = np.ascontiguousarray(prep["xTf"][c * BL:(c + 1) * BL])
        im["xTb"] = np.ascontiguousarray(prep["xTb"][c * BL:(c + 1) * BL])
        in_maps.append(im)

    res = run_bass_kernel_spmd(nc, in_maps, list(range(NCORES)),
                               **_BUILD_CACHE.get("run_kwargs", {}))
    _BUILD_CACHE["last_results"] = res
    outs = [res.results[c]["outT"].transpose(0, 2, 1) for c in range(NCORES)]
    return np.ascontiguousarray(np.concatenate(outs, 0).astype(np.float32))



# revision 52
# speedup vs baseline: 1.3585x; 1.1398x over previous
"""Trainium2 Bass kernel for nn_MixedAttentionModule (CvT-style mixed attention block).

Data-parallel over batch: 32 batches -> 8 cores x 4 batches. No collectives.
All layouts channel-major on device (activations [C, n]); host pre-transposes x
and post-transposes the output. LN/BN/bias affines are folded into adjacent
weights on the host. Depthwise 3x3 convs run on the tensor engine as 9
diagonal matmuls accumulating in PSUM. Attention computes scores^T = k q^T so
the softmax denominator is a ones-matmul and attn@v needs no transpose.
"""
import sys

sys.path.insert(0, "/opt/trn_rl_repo")

import numpy as np
import ml_dtypes

B, n, C, NH, HD, FF = 32, 1024, 768, 12, 64, 3072
Ht = Wt = 32
M = 256          # kv positions (16*16)
NCORES = 8
BL = B // NCORES  # batches per core
EPS = 1e-5
KT = C // 128     # 6 channel tiles
FT = FF // 128    # 24 ff tiles
NCH = 2           # n-chunks of 512
SX = 8.0          # fp8 scale on LN2 output (|ln| <= sqrt(C)=27.7, *8 = 222 < 240)
LN_SX = 2.0794415416798357   # ln(SX), folded into the rstd exp
SV = 16.0         # fp8 scale on v tokens (|v| ~ 0.8, *16 = 13 << 240)
NLN_SV = -2.772588722239781  # -ln(SV), folded into the sinv exp
F32 = None
BF16 = None

_BUILD_CACHE = {}


def _patch_compiler(ldw_opt=True):
    """Patch bass' walrus invocation: keep the standard pass list but allow
    toggling the LDWEIGHTS-dedup codegen optimization."""
    from pathlib import Path
    from concourse import bass_utils

    def patched(tmpdir, inp="bir.json", outp="file.neff", arch=None, *, dve_root=None):
        cmd = [
            bass_utils.get_walrus_driver(),
            "--pass",
            "birverifier,runtime_memory_reservation,lower_act,lower_dve,"
            "lower_ap_offset,codegen,neff_packager",
            "-i", inp,
            "--neff-output-filename", outp,
            "--enable-birsim=true",
            "--mem-mode=physical",
            "--policy=0",
            f"--enable-ldw-opt={'true' if ldw_opt else 'false'}",
            "--assign-static-dmas-to-sp=false",
            f"--dram-page-size={bass_utils.aot_getenv('NEURON_SCRATCHPAD_PAGE_SIZE', '256')}",
            "--enable-neff-debug-info=true",
            "--jobs", "8",
            *bass_utils.get_walrus_args(
                bass_utils.get_bir_arch(tmpdir, inp) if arch is None else arch,
                tmpdir, dve_root=dve_root,
            ),
        ]
        result = bass_utils.run_command(cmd, cwd=tmpdir)
        if result is not None:
            (Path(tmpdir) / "log.txt").write_text(result.stdout)
        return f"{tmpdir}/{outp}"

    bass_utils.bir_verify_and_optimise = patched


def _split_sync_waits(nc, max_waits=1):
    """walrus codegen in this environment allows at most one sync wait per
    instruction. Hoist excess waits onto standalone EventSemaphore carriers
    inserted just before, on the same engine (engines execute their stream
    in order, so this is equivalent)."""
    from concourse import mybir

    n_new = 0
    for f in nc.m.functions:
        for blk in f.blocks:
            out = []
            for inst in blk.instructions:
                si = getattr(inst, "sync_info", None)
                if si is not None:
                    waits = list(si.on_wait or [])
                    ups = list(si.on_update or [])
                    if len(waits) > max_waits:
                        extra = waits[: len(waits) - max_waits]
                        keep = waits[len(waits) - max_waits:]
                        for w in extra:
                            n_new += 1
                            out.append(mybir.InstEventSemaphore(
                                name=f"syncw-{n_new}-{inst.name}",
                                ins=[], outs=[],
                                engine=inst.engine,
                                sync_info=mybir.SyncInfo(on_wait=[w], on_update=[]),
                            ))
                        inst.sync_info = mybir.SyncInfo(on_wait=keep, on_update=ups)
                out.append(inst)
            blk.instructions = out
    return n_new


def _build_program():
    from concourse import bass, mybir, tile

    f32 = mybir.dt.float32
    bf16 = mybir.dt.bfloat16
    Alu = mybir.AluOpType
    Act = mybir.ActivationFunctionType
    DRM = mybir.MatmulPerfMode.DoubleRow

    f8 = mybir.dt.float8e4

    nc = bass.Bass("TRN2", target_bir_lowering=False, debug=False, num_devices=NCORES)

    # ---- DRAM I/O ----
    xTf = nc.dram_tensor("xTf", [BL, C, n], f32, kind="ExternalInput").ap()
    xTb = nc.dram_tensor("xTb", [BL, C, n], bf16, kind="ExternalInput").ap()
    # fp8 x and x^2, packed [p, k*n+t] for DoubleRow LN1 stats
    x8p = nc.dram_tensor("x8p", [BL, 128, KT * n], f8, kind="ExternalInput").ap()
    xsq8 = nc.dram_tensor("xsq8", [BL, 128, KT * n], f8, kind="ExternalInput").ap()
    # fp8 projection weights packed for DoubleRow:
    #   wq8/wk8[p, mt*768 + kp*256 + i*128 + m] = W_eff[mt*128+m, (2kp+i)*128+p]*s
    #   wv8[p, k*768 + c] = Wv_eff[c, k*128+p]*s   (moving operand)
    wq8 = nc.dram_tensor("wq8", [128, KT * C], f8, kind="ExternalInput").ap()
    wk8 = nc.dram_tensor("wk8", [128, KT * C], f8, kind="ExternalInput").ap()
    wv8 = nc.dram_tensor("wv8", [128, KT * C], f8, kind="ExternalInput").ap()
    scq_d = nc.dram_tensor("scq", [128, 1], f32, kind="ExternalInput").ap()
    sck_d = nc.dram_tensor("sck", [128, 1], f32, kind="ExternalInput").ap()
    scv_d = nc.dram_tensor("scv", [128, 1], f32, kind="ExternalInput").ap()
    # fp8 W1 packed for DoubleRow; bf16 W2 packed per ft-tile:
    #   w1q[p, ft*768 + kp*256 + i*128 + f] = W1eff[ft*128+f, (2kp+i)*128+p] * s1
    #   w2r[p, ft*768 + mt*128 + m] = W2[mt*128+m, ft*128+p]
    w1q = nc.dram_tensor("w1q", [128, FT * C], f8, kind="ExternalInput").ap()
    w2r = nc.dram_tensor("w2r", [128, FT * C], bf16, kind="ExternalInput").ap()
    sc1_d = nc.dram_tensor("sc1", [128, 1], f32, kind="ExternalInput").ap()
    dq9 = nc.dram_tensor("dq9", [KT, 128, 9 * 128], bf16, kind="ExternalInput").ap()
    dk9 = nc.dram_tensor("dk9", [KT, 128, 9 * 128], bf16, kind="ExternalInput").ap()
    dv9 = nc.dram_tensor("dv9", [KT, 128, 9 * 128], bf16, kind="ExternalInput").ap()
    bq_d = nc.dram_tensor("bq", [C, 1], f32, kind="ExternalInput").ap()
    bk_d = nc.dram_tensor("bk", [C, 1], f32, kind="ExternalInput").ap()
    bva_d = nc.dram_tensor("bva", [C, 1], f32, kind="ExternalInput").ap()
    b1_d = nc.dram_tensor("b1", [FF, 1], f32, kind="ExternalInput").ap()
    ones_sq_d = nc.dram_tensor("ones_sq", [128, 128], bf16, kind="ExternalInput").ap()
    outT = nc.dram_tensor("outT", [BL, C, n], f32, kind="ExternalOutput").ap()

    with tile.TileContext(nc) as tc:
        with tc.tile_pool(name="P", bufs=1) as P:
            # ---- persistent SBUF (weights + per-batch activations) ----
            wq_sb = P.tile([128, KT * C], f8, name="wq8", tag="wq", bufs=1)
            wk_sb = P.tile([128, KT * C], f8, name="wk8", tag="wk", bufs=1)
            wv_sb = P.tile([128, KT * C], f8, name="wv8", tag="wv", bufs=1)
            bq6 = P.tile([128, KT], f32, name="bq6", tag="bq", bufs=1)
            bk6 = P.tile([128, KT], f32, name="bk6", tag="bk", bufs=1)
            bva6 = P.tile([128, KT], f32, name="bva6", tag="bva", bufs=1)
            b1_24 = P.tile([128, FT], f32, name="b1_24", tag="b1", bufs=1)
            ones_sq = P.tile([128, 128], bf16, name="onessq", tag="onessq", bufs=1)
            w1_sb = P.tile([128, FT * C], f8, name="w1q", tag="w1q", bufs=1)
            sc1_sb = P.tile([128, 1], f32, name="sc1", tag="sc1", bufs=1)
            scq_sb = P.tile([128, 1], f32, name="scq", tag="scq", bufs=1)
            sck_sb = P.tile([128, 1], f32, name="sck", tag="sck", bufs=1)
            scv_sb = P.tile([128, 1], f32, name="scv", tag="scv", bufs=1)

            # ones first (LN1 stats needs them immediately); biases arrive as
            # one packed strided DMA per tensor; heavy projection weights go on
            # the gpsimd queue so batch 0's x isn't stuck behind them.
            nc.sync.dma_start(ones_sq[:], ones_sq_d[:, :])
            nc.sync.dma_start(bq6[:], bq_d.rearrange("(t p) o -> p (t o)", p=128))
            nc.sync.dma_start(bk6[:], bk_d.rearrange("(t p) o -> p (t o)", p=128))
            nc.sync.dma_start(bva6[:], bva_d.rearrange("(t p) o -> p (t o)", p=128))
            nc.sync.dma_start(b1_24[:], b1_d.rearrange("(t p) o -> p (t o)", p=128))
            nc.sync.dma_start(sc1_sb[:], sc1_d[:, :])
            nc.sync.dma_start(scq_sb[:], scq_d[:, :])
            nc.sync.dma_start(sck_sb[:], sck_d[:, :])
            nc.sync.dma_start(scv_sb[:], scv_d[:, :])
            nc.gpsimd.dma_start(wq_sb[:], wq8[:, :])
            nc.gpsimd.dma_start(wk_sb[:], wk8[:, :])
            nc.gpsimd.dma_start(wv_sb[:], wv8[:, :])

            pad = [P.tile([128, 34, 34], bf16, name=f"pad{k}", tag="pad", bufs=KT) for k in range(KT)]
            xb_sb = [P.tile([128, n], bf16, name=f"xb{k}", tag="xb", bufs=KT) for k in range(KT)]
            xb8_sb = P.tile([128, KT * n], f8, name="xb8", tag="xb8", bufs=1)
            xsq8_sb = P.tile([128, KT * n], f8, name="xsq8", tag="xsq8", bufs=1)
            act8q = P.tile([128, KT * n], f8, name="a8q", tag="aq", bufs=1)
            act8k = P.tile([128, KT * M], f8, name="a8k", tag="ak", bufs=1)
            act8v = P.tile([128, KT * M], f8, name="a8v", tag="av", bufs=1)
            qT = [P.tile([128, n], bf16, name=f"qT{k}", tag="qT", bufs=KT) for k in range(KT)]
            kTt = [P.tile([128, M], bf16, name=f"kT{k}", tag="kT", bufs=KT) for k in range(KT)]
            vt8 = P.tile([128, 2, C], f8, name="vt8", tag="vt", bufs=1)
            ones8 = P.tile([128, 256], f8, name="ones8", tag="ones8", bufs=1)
            nc.vector.memset(ones8[:], 1.0)
            # half-zeroed ones stationaries: accumulate both heads' kv-sums into
            # disjoint partition halves of one PSUM tile
            ones_hf = [P.tile([128, 256], f8, name=f"oneshf{hh}", tag="oneshf", bufs=2)
                       for hh in range(2)]
            for hh in range(2):
                nc.vector.memset(ones_hf[hh][:], 0.0)
                nc.vector.memset(ones_hf[hh][:, hh * 64:hh * 64 + 64], 1.0)
                nc.vector.memset(ones_hf[hh][:, 128 + hh * 64:128 + hh * 64 + 64], 1.0)
            OT = [P.tile([128, n], bf16, name=f"OT{k}", tag="OT", bufs=KT) for k in range(KT)]
            x2b = [P.tile([128, n], bf16, name=f"x2{k}", tag="x2", bufs=KT) for k in range(KT)]
            # LN2 output: fp8, all 6 channel tiles in one buffer so DoubleRow can
            # pair adjacent k-tiles along the free dim (stride n between planes)
            xl8 = P.tile([128, KT * n], f8, name="xl8", tag="xl8", bufs=1)

            # zero the padded conv buffers once (interiors are overwritten per batch;
            # the one-element borders must stay zero)
            for k in range(KT):
                nc.vector.memset(pad[k][:], 0.0)

            # constant APs for float biases of activation ops
            czero = P.tile([128, 1], f32, name="czero", tag="cz", bufs=2)
            nc.vector.memset(czero[:], 0.0)
            nc.const_aps.aps[(f32, 0.0)] = czero[:]
            ceps = P.tile([128, 1], f32, name="ceps", tag="cz", bufs=2)
            nc.vector.memset(ceps[:], EPS)
            nc.const_aps.aps[(f32, EPS)] = ceps[:]
            cln8 = P.tile([128, 1], f32, name="cln8", tag="cln8", bufs=1)
            nc.vector.memset(cln8[:], LN_SX)
            nc.const_aps.aps[(f32, LN_SX)] = cln8[:]
            cnv = P.tile([128, 1], f32, name="cnv", tag="cnv", bufs=1)
            nc.vector.memset(cnv[:], NLN_SV)
            nc.const_aps.aps[(f32, NLN_SV)] = cnv[:]

            def ln_finish(ps_mean, ps_msq, src_of, dst_write, label, bno, rstd_bias):
                for ch in range(NCH):
                    sl = slice(ch * 512, (ch + 1) * 512)
                    mbc = P.tile([128, 512], f32, name=f"mbc{label}{bno}_{ch}", tag="mbc", bufs=2)
                    rbc = P.tile([128, 512], f32, name=f"rbc{label}{bno}_{ch}", tag="rbc", bufs=2)
                    nc.vector.tensor_scalar_mul(mbc[:], ps_mean[ch][:], 1.0 / C)
                    # rstd = 1/sqrt((msq/C) - mean^2 + eps)
                    nc.vector.tensor_mul(rbc[:], mbc[:], mbc[:])
                    nc.vector.scalar_tensor_tensor(rbc[:], ps_msq[ch][:], 1.0 / C,
                                                   rbc[:], Alu.mult, Alu.subtract)
                    # rstd = exp(-0.5*ln(var+eps)) on ACT (keeps DVE free;
                    # table accuracy ~1e-4 rel, far below bf16 noise);
                    # rstd_bias multiplies rstd by exp(rstd_bias) for free
                    nc.scalar.activation(rbc[:], rbc[:], Act.Ln, bias=EPS)
                    nc.scalar.activation(rbc[:], rbc[:], Act.Exp, scale=-0.5,
                                         bias=rstd_bias)
                    for k in range(KT):
                        dst_write(k, ch, src_of(k, sl), mbc, rbc)

            ones8v = ones8[:].rearrange("p (i f) -> p i f", i=2)

            def ln_stats_apply(src_tiles, dst_write, label, bno, rstd_bias=0.0):
                """src_tiles: 6 bf16 [128, n] channel-major tiles.
                Stats matmuls use a ones[128,128] stationary so the channel-sums
                arrive pre-broadcast across all 128 partitions; all row math is
                then full-width DVE work and no PE broadcast is needed."""
                with tc.tile_pool(name=f"ps_ln_{label}{bno}", bufs=1, space="PSUM") as psp:
                    ps_mean = [psp.tile([128, 512], f32, name=f"psm{label}{bno}_{c}", tag="mm", bufs=4) for c in range(NCH)]
                    ps_msq = [psp.tile([128, 512], f32, name=f"psq{label}{bno}_{c}", tag="mm", bufs=4) for c in range(NCH)]
                    # squares on ACT (bf16 out), then ones-matmul stats; groups are
                    # interleaved across banks so sq tiles can double-buffer
                    for k in range(KT):
                        sqt = P.tile([128, n], bf16, name=f"sq{label}{bno}_{k}", tag="sq", bufs=2)
                        nc.scalar.activation(sqt[:], src_tiles[k][:], Act.Square)
                        for ch in range(NCH):
                            sl = slice(ch * 512, (ch + 1) * 512)
                            nc.tensor.matmul(ps_mean[ch][:], ones_sq[:], src_tiles[k][:, sl],
                                             start=(k == 0), stop=(k == KT - 1))
                            nc.tensor.matmul(ps_msq[ch][:], ones_sq[:], sqt[:, sl],
                                             start=(k == 0), stop=(k == KT - 1))
                    ln_finish(ps_mean, ps_msq, lambda k, sl: src_tiles[k][:, sl],
                              dst_write, label, bno, rstd_bias)

            def ln1_stats_apply(dst_write, bno):
                """LN1 stats from host-precomputed fp8 x and x^2 via DoubleRow
                ones-matmuls (half the PE passes, no device squares)."""
                with tc.tile_pool(name=f"ps_ln_a{bno}", bufs=1, space="PSUM") as psp:
                    ps_mean = [psp.tile([128, 512], f32, name=f"psma{bno}_{c}", tag="mm", bufs=4) for c in range(NCH)]
                    ps_msq = [psp.tile([128, 512], f32, name=f"psqa{bno}_{c}", tag="mm", bufs=4) for c in range(NCH)]
                    for kp in range(KT // 2):
                        for ch in range(NCH):
                            sl = slice(ch * 512, (ch + 1) * 512)
                            xap = xb8_sb[:, 2 * kp * n:(2 * kp + 2) * n] \
                                .rearrange("p (i t) -> p i t", i=2)[:, :, sl]
                            sap = xsq8_sb[:, 2 * kp * n:(2 * kp + 2) * n] \
                                .rearrange("p (i t) -> p i t", i=2)[:, :, sl]
                            nc.tensor.matmul(ps_mean[ch][:], ones8v, xap,
                                             start=(kp == 0), stop=(kp == KT // 2 - 1),
                                             perf_mode=DRM)
                            nc.tensor.matmul(ps_msq[ch][:], ones8v, sap,
                                             start=(kp == 0), stop=(kp == KT // 2 - 1),
                                             perf_mode=DRM)
                    ln_finish(ps_mean, ps_msq, lambda k, sl: xb_sb[k][:, sl],
                              dst_write, "a", bno, 0.0)

            for b in range(BL):
                # ---------------- LN1 + conv + projections ----------------
                for k in range(KT):
                    nc.sync.dma_start(xb_sb[k][:], xTb[b, k * 128:(k + 1) * 128, :])
                nc.sync.dma_start(xb8_sb[:], x8p[b, :, :])
                nc.sync.dma_start(xsq8_sb[:], xsq8[b, :, :])

                def ln1_write(k, ch, src, mbc, rbc):
                    tmp = P.tile([128, 512], f32, name=f"t1_{b}_{k}_{ch}", tag="tmp", bufs=3)
                    nc.vector.tensor_sub(tmp[:], src, mbc[:])
                    # write normalized values into padded interior rows
                    r0 = 1 + 16 * ch
                    dst = pad[k][:, r0:r0 + 16, 1:33]
                    nc.vector.tensor_mul(dst, tmp[:].rearrange("p (a c) -> p a c", a=16), rbc[:].rearrange("p (a c) -> p a c", a=16))
                    return

                ln1_stats_apply(ln1_write, b)

                # conv (9 diagonal matmuls per output chunk) + exact ELU
                with tc.tile_pool(name=f"ps_conv{b}", bufs=1, space="PSUM") as cvp:
                    def elu_chain(ps_ap, dst_ap, width):
                        tmin = P.tile([128, width], f32, name=f"tm{b}", tag="tmpe", bufs=3)
                        et = P.tile([128, width], bf16, name=f"ee{b}", tag="ee", bufs=3)
                        nc.vector.tensor_scalar_min(tmin[:], ps_ap, 0.0)
                        nc.scalar.activation(et[:], tmin[:], Act.Exp)
                        # elu+1 = relu(x) + exp(min(x,0));  the -1 is folded into proj biases
                        nc.vector.scalar_tensor_tensor(dst_ap, ps_ap, 0.0, et[:], Alu.max, Alu.add)

                    for k in range(KT):
                        dqt = P.tile([128, 9 * 128], bf16, name=f"dq{b}_{k}", tag="dq", bufs=2)
                        nc.gpsimd.dma_start(dqt[:], dq9[k, :, :])
                        pq = [cvp.tile([128, 512], f32, name=f"pcq{b}_{k}_{c}", tag="mm", bufs=4) for c in range(NCH)]
                        for tap in range(9):
                            dy, dx = tap // 3, tap % 3
                            for ch in range(NCH):
                                rhs = pad[k][:, dy + 16 * ch:dy + 16 * ch + 16, dx:dx + 32]
                                nc.tensor.matmul(pq[ch][:], dqt[:, tap * 128:(tap + 1) * 128], rhs,
                                                 start=(tap == 0), stop=(tap == 8))
                        for ch in range(NCH):
                            elu_chain(pq[ch][:], act8q[:, k * n + ch * 512:k * n + (ch + 1) * 512], 512)
                    for k in range(KT):
                        dkt = P.tile([128, 9 * 128], bf16, name=f"dk{b}_{k}", tag="dkv", bufs=2)
                        dvt = P.tile([128, 9 * 128], bf16, name=f"dv{b}_{k}", tag="dkv", bufs=2)
                        nc.gpsimd.dma_start(dkt[:], dk9[k, :, :])
                        nc.gpsimd.dma_start(dvt[:], dv9[k, :, :])
                        pk = cvp.tile([128, M], f32, name=f"pck{b}_{k}", tag="mm", bufs=4)
                        pv = cvp.tile([128, M], f32, name=f"pcv{b}_{k}", tag="mm", bufs=4)
                        for tap in range(9):
                            dy, dx = tap // 3, tap % 3
                            rhs = pad[k][:, dy:dy + 32:2, dx:dx + 32:2]
                            nc.tensor.matmul(pk[:], dkt[:, tap * 128:(tap + 1) * 128], rhs,
                                             start=(tap == 0), stop=(tap == 8))
                            nc.tensor.matmul(pv[:], dvt[:, tap * 128:(tap + 1) * 128], rhs,
                                             start=(tap == 0), stop=(tap == 8))
                        elu_chain(pk[:], act8k[:, k * M:(k + 1) * M], M)
                        elu_chain(pv[:], act8v[:, k * M:(k + 1) * M], M)

                    # projections -- fp8 DoubleRow over contraction-tile pairs;
                    # dequant scale + bias applied in one DVE tensor_scalar
                    def a8pair(act8, width, kp, sl2):
                        return act8[:, 2 * kp * width:(2 * kp + 2) * width] \
                            .rearrange("p (i t) -> p i t", i=2)[:, :, sl2]

                    for mt in range(KT):
                        psq = [cvp.tile([128, 512], f32, name=f"pq{b}_{mt}_{c}", tag="mm", bufs=4)
                               for c in range(NCH)]
                        for kp in range(KT // 2):
                            wap = wq_sb[:, mt * C + kp * 256:mt * C + (kp + 1) * 256] \
                                .rearrange("p (i f) -> p i f", i=2)
                            for ch in range(NCH):
                                nc.tensor.matmul(psq[ch][:], wap,
                                                 a8pair(act8q, n, kp, slice(ch * 512, (ch + 1) * 512)),
                                                 start=(kp == 0), stop=(kp == KT // 2 - 1),
                                                 perf_mode=DRM)
                        for ch in range(NCH):
                            nc.vector.tensor_scalar(qT[mt][:, ch * 512:(ch + 1) * 512], psq[ch][:],
                                                    scq_sb[:], bq6[:, mt:mt + 1],
                                                    Alu.mult, Alu.add)
                    for mt in range(KT):
                        ps = cvp.tile([128, M], f32, name=f"pk{b}_{mt}", tag="mm", bufs=4)
                        for kp in range(KT // 2):
                            wap = wk_sb[:, mt * C + kp * 256:mt * C + (kp + 1) * 256] \
                                .rearrange("p (i f) -> p i f", i=2)
                            nc.tensor.matmul(ps[:], wap, a8pair(act8k, M, kp, slice(0, M)),
                                             start=(kp == 0), stop=(kp == KT // 2 - 1),
                                             perf_mode=DRM)
                        nc.vector.tensor_scalar(kTt[mt][:, :], ps[:],
                                                sck_sb[:], bk6[:, mt:mt + 1],
                                                Alu.mult, Alu.add)
                    for mt2 in range(2):
                        psv = [cvp.tile([128, w], f32, name=f"pv{b}_{mt2}_{c}", tag="mm", bufs=4)
                               for c, w in [(0, 512), (1, 256)]]
                        for kp in range(KT // 2):
                            aap = a8pair(act8v, M, kp, slice(mt2 * 128, (mt2 + 1) * 128))
                            for ch, w in [(0, 512), (1, 256)]:
                                nc.tensor.matmul(psv[ch][:], aap,
                                                 wv_sb[:, 2 * kp * C:(2 * kp + 2) * C]
                                                 .rearrange("p (i c) -> p i c", i=2)[:, :, ch * 512:ch * 512 + w],
                                                 start=(kp == 0), stop=(kp == KT // 2 - 1),
                                                 perf_mode=DRM)
                        for ch, w in [(0, 512), (1, 256)]:
                            # v tokens in fp8, prescaled by SV/s_wv (folded out via sinv)
                            nc.vector.tensor_scalar(
                                vt8[:, mt2:mt2 + 1, ch * 512:ch * 512 + w], psv[ch][:],
                                scv_sb[:], None, Alu.mult)

                if b == 0:
                    # one-time fp8 W1 load; queued here so batch 0's conv
                    # weights (same gpsimd queue) aren't delayed behind it
                    for half in range(4):
                        slh = slice(half * (FT * C // 4), (half + 1) * (FT * C // 4))
                        nc.gpsimd.dma_start(w1_sb[:, slh], w1q[:, slh])

                # ---------------- attention ----------------
                # software-pipelined over head pairs: scores(j+1) are emitted
                # before sum/AV(j) so the PE streams while ACT runs the exps
                with tc.tile_pool(name=f"ps_at{b}", bufs=1, space="PSUM") as atp:
                    def att_scores(j):
                        # exp(scores) in fp8, kv tiles stacked for DoubleRow;
                        # alternate the two heads' row-halves so the PE streams
                        # both halves concurrently
                        ET2 = [P.tile([128, 2, n], f8, name=f"ET{b}_{j}_{hh}", tag="ET", bufs=4)
                               for hh in range(2)]
                        for mt in range(2):
                            for ch in range(NCH):
                                for hh in range(2):
                                    bp = 64 * hh
                                    ps = atp.tile([128, 512], f32, name=f"pss{b}_{j}_{hh}_{mt}_{ch}", tag="mm", bufs=4)
                                    nc.tensor.matmul(ps[:],
                                                     kTt[j][bp:bp + 64, mt * 128:(mt + 1) * 128],
                                                     qT[j][bp:bp + 64, ch * 512:(ch + 1) * 512],
                                                     tile_position=(bp, 0))
                                    nc.scalar.activation(ET2[hh][:, mt:mt + 1, ch * 512:(ch + 1) * 512],
                                                         ps[:], Act.Exp, scale=0.125)
                        return ET2

                    def att_finish(j, ET2):
                        # kv-sums of both heads accumulate into disjoint partition
                        # halves of one PSUM tile (half-zeroed ones stationaries),
                        # so Ln/Exp run once per chunk at full width
                        sinv = [P.tile([128, 512], f32, name=f"si{b}_{j}_{c}", tag="sinv", bufs=4)
                                for c in range(NCH)]
                        for ch in range(NCH):
                            sum_ps = atp.tile([128, 512], f32, name=f"psum{b}_{j}_{ch}", tag="bc", bufs=2)
                            for hh in range(2):
                                nc.tensor.matmul(sum_ps[:],
                                                 ones_hf[hh][:].rearrange("p (i f) -> p i f", i=2),
                                                 ET2[hh][:, :, ch * 512:(ch + 1) * 512],
                                                 start=(hh == 0), stop=(hh == 1),
                                                 perf_mode=DRM)
                            # 1/(s*SV) = exp(-ln(s) - ln SV) on ACT
                            nc.scalar.activation(sinv[ch][:], sum_ps[:], Act.Ln)
                            nc.scalar.activation(sinv[ch][:], sinv[ch][:], Act.Exp,
                                                 scale=-1.0, bias=NLN_SV)
                        po = [atp.tile([128, 512], f32, name=f"po{b}_{j}_{c}", tag="o", bufs=2)
                              for c in range(NCH)]
                        for mt in range(2):
                            for ch in range(NCH):
                                for hh in range(2):
                                    bp = 64 * hh
                                    h = 2 * j + hh
                                    nc.tensor.matmul(po[ch][bp:bp + 64, :],
                                                     vt8[:, mt:mt + 1, h * 64:(h + 1) * 64],
                                                     ET2[hh][:, mt:mt + 1, ch * 512:(ch + 1) * 512],
                                                     start=(mt == 0), stop=(mt == 1),
                                                     tile_position=(0, bp))
                        for ch in range(NCH):
                            sl = slice(ch * 512, (ch + 1) * 512)
                            nc.vector.tensor_mul(OT[j][:, sl], po[ch][:], sinv[ch][:])

                    ET_prev = att_scores(0)
                    for j in range(1, NH // 2):
                        ET_cur = att_scores(j)
                        att_finish(j - 1, ET_prev)
                        ET_prev = ET_cur
                    att_finish(NH // 2 - 1, ET_prev)

                # ---------------- residual + LN2 ----------------
                for k in range(KT):
                    for ch in range(NCH):
                        sl = slice(ch * 512, (ch + 1) * 512)
                        xf = P.tile([128, 512], f32, name=f"xf{b}_{k}_{ch}", tag="xf", bufs=3)
                        nc.sync.dma_start(xf[:], xTf[b, k * 128:(k + 1) * 128, sl])
                        nc.vector.scalar_tensor_tensor(x2b[k][:, sl], OT[k][:, sl], bva6[:, k:k + 1], xf[:],
                                                       Alu.add, Alu.add)

                def ln2_write(k, ch, src, mbc, rbc):
                    tmp = P.tile([128, 512], f32, name=f"t2_{b}_{k}_{ch}", tag="tmp", bufs=3)
                    nc.vector.tensor_sub(tmp[:], src, mbc[:])
                    # rbc carries exp(ln 8) = SX, so this writes xn*8 in fp8e4
                    nc.vector.tensor_mul(xl8[:, k * n + ch * 512:k * n + (ch + 1) * 512],
                                         tmp[:], rbc[:])

                ln_stats_apply(x2b, ln2_write, "c", b, rstd_bias=LN_SX)

                # ---------------- FFN (fp8 DoubleRow h1, bf16 h2) + residual ----------------
                # software-pipelined: h1(ft+1) is emitted before h2(ft) so the
                # PE streams through the gelu latency
                with tc.tile_pool(name=f"ps_ffn{b}", bufs=1, space="PSUM") as ffp:
                    for ch in range(NCH):
                        sl = slice(ch * 512, (ch + 1) * 512)
                        ph2 = [ffp.tile([128, 512], f32, name=f"ph2_{b}_{ch}_{mt}", tag="h2", bufs=6)
                               for mt in range(KT)]
                        ph1s, w2bs = {}, {}

                        def emit_h1(ft):
                            w2b = P.tile([128, C], bf16, name=f"w2_{b}_{ch}_{ft}", tag="w2", bufs=3)
                            nc.gpsimd.dma_start(w2b[:], w2r[:, ft * C:(ft + 1) * C])
                            w2bs[ft] = w2b
                            ph1 = ffp.tile([128, 512], f32, name=f"ph1_{b}_{ch}_{ft}", tag="h1", bufs=2)
                            for kp in range(KT // 2):
                                w1ap = w1_sb[:, ft * C + kp * 256: ft * C + (kp + 1) * 256] \
                                    .rearrange("p (i f) -> p i f", i=2)
                                xap = xl8[:, 2 * kp * n:(2 * kp + 2) * n] \
                                    .rearrange("p (i t) -> p i t", i=2)[:, :, sl]
                                nc.tensor.matmul(ph1[:], w1ap, xap,
                                                 start=(kp == 0), stop=(kp == KT // 2 - 1),
                                                 perf_mode=DRM)
                            ph1s[ft] = ph1

                        emit_h1(0)
                        for ft in range(FT):
                            if ft + 1 < FT:
                                emit_h1(ft + 1)
                            gt = P.tile([128, 512], bf16, name=f"g_{b}_{ch}_{ft}", tag="g", bufs=3)
                            nc.scalar.activation(gt[:], ph1s.pop(ft)[:], Act.Gelu,
                                                 bias=b1_24[:, ft:ft + 1], scale=sc1_sb[:])
                            w2b = w2bs.pop(ft)
                            for mt in range(KT):
                                nc.tensor.matmul(ph2[mt][:],
                                                 w2b[:, mt * 128:(mt + 1) * 128],
                                                 gt[:],
                                                 start=(ft == 0), stop=(ft == FT - 1))
                        for mt in range(KT):
                            xf2 = P.tile([128, 512], f32, name=f"xf2_{b}_{ch}_{mt}", tag="xf", bufs=3)
                            nc.sync.dma_start(xf2[:], xTf[b, mt * 128:(mt + 1) * 128, sl])
                            ub = P.tile([128, 512], f32, name=f"u_{b}_{ch}_{mt}", tag="tmp", bufs=3)
                            nc.vector.scalar_tensor_tensor(ub[:], OT[mt][:, sl], bva6[:, mt:mt + 1], xf2[:],
                                                           Alu.add, Alu.add)
                            ob = P.tile([128, 512], f32, name=f"o_{b}_{ch}_{mt}", tag="ob", bufs=3)
                            nc.vector.tensor_add(ob[:], ub[:], ph2[mt][:])
                            nc.sync.dma_start(outT[b, mt * 128:(mt + 1) * 128, sl], ob[:])
    n_hoisted = _split_sync_waits(nc)
    print(f"_split_sync_waits: hoisted waits onto {n_hoisted} carrier instructions")
    return nc


def _host_prep(inputs):
    """Fold LN/BN affines into weights; build packed bf16 arrays."""
    f = lambda k: np.asarray(inputs[k], np.float32)
    bfc = lambda a: np.ascontiguousarray(a.astype(ml_dtypes.bfloat16))
    x = f("x")                         # (B, n, C)
    ln1_g, ln1_b = f("ln1_g"), f("ln1_b")
    ln2_g, ln2_b = f("ln2_g"), f("ln2_b")

    f8c = lambda a: np.clip(a, -240.0, 240.0).astype(ml_dtypes.float8_e4m3)
    prep = {}
    xT = np.ascontiguousarray(x.transpose(0, 2, 1))   # (B, C, n)
    prep["xTf"] = xT
    prep["xTb"] = bfc(xT)
    # fp8 x and x^2 packed [b, p, k*n + t] for DoubleRow LN1 stats
    xp = xT.reshape(B, KT, 128, Ht * Wt).transpose(0, 2, 1, 3).reshape(B, 128, KT * Ht * Wt)
    prep["x8p"] = np.ascontiguousarray(f8c(xp))
    prep["xsq8"] = np.ascontiguousarray(f8c(xp * xp))

    diag9 = {}
    badj = {}
    for nm in ["q", "k", "v"]:
        w = f(f"dw_w_{nm}")[:, 0]                     # (C,3,3)
        w_eff = w * ln1_g[:, None, None]
        cb = f(f"dw_b_{nm}") + ln1_b * w.sum((1, 2))  # exact only if ln1_b == 0 (boundary)
        assert np.abs(cb).max() < 1e-30, "nonzero conv bias not implemented on device"
        sc = f(f"bn_g_{nm}") / np.sqrt(f(f"bn_v_{nm}") + EPS)
        sh = f(f"bn_b_{nm}") - f(f"bn_m_{nm}") * sc
        W = f(f"W_{nm}")
        W_eff = W * sc[None, :]
        s_w = 2.0 ** np.floor(np.log2(224.0 / max(np.abs(W_eff).max(), 1e-30)))
        # the device multiplies with the fp8 weights, so the elu+1 "-1" fold
        # must subtract the row sums of the QUANTIZED weights or a constant
        # per-channel offset (Wq-W).sum(1) leaks into the output
        W_deq = f8c(W_eff * s_w).astype(np.float32) / s_w
        b_eff = f(f"b_{nm}") + W @ sh - W_deq.sum(1)
        # pack 9 taps of diagonal matrices: [KT, 128, 9*128]
        d = np.zeros((KT, 128, 9 * 128), np.float32)
        for kt in range(KT):
            ww = w_eff[kt * 128:(kt + 1) * 128]       # (128,3,3)
            for tap in range(9):
                dy, dx = tap // 3, tap % 3
                d[kt, np.arange(128), tap * 128 + np.arange(128)] = ww[:, dy, dx]
        diag9[nm] = bfc(d)
        badj[nm] = b_eff
        if nm == "v":
            # moving operand: wv8[p, k*768 + c] = W_eff.T[k*128+p, c] * s
            wv = (W_eff.T * s_w).reshape(KT, 128, C).transpose(1, 0, 2).reshape(128, KT * C)
            prep["wv8"] = np.ascontiguousarray(f8c(wv))
            prep["scv"] = np.full((128, 1), SV / s_w, np.float32)
        else:
            # stationary: w8[p, mt*768 + kp*256 + i*128 + m] = W_eff[mt*128+m, (2kp+i)*128+p]*s
            wq = (W_eff * s_w).reshape(KT, 128, KT, 128).transpose(3, 0, 2, 1).reshape(128, KT * C)
            prep[f"w{nm}8"] = np.ascontiguousarray(f8c(wq))
            prep[f"sc{nm}"] = np.full((128, 1), 1.0 / s_w, np.float32)
    prep["dq9"], prep["dk9"], prep["dv9"] = diag9["q"], diag9["k"], diag9["v"]
    prep["bq"] = badj["q"].reshape(C, 1)
    prep["bk"] = badj["k"].reshape(C, 1)
    prep["bva"] = badj["v"].reshape(C, 1)

    W1 = f("W1") * ln2_g[None, :]                     # (FF, C)
    b1 = f("b1") + f("W1") @ ln2_b
    W2 = f("W2")                                      # (C, FF)
    assert np.abs(f("b2")).max() < 1e-30, "nonzero b2 not implemented on device"
    # fp8e4 (TRN: max +-240) DoubleRow packing, power-of-2 per-tensor scale
    s1 = 2.0 ** np.floor(np.log2(224.0 / max(np.abs(W1).max(), 1e-30)))
    # w1q[p, ft*768 + kp*256 + i*128 + f] = W1[ft*128+f, (2kp+i)*128+p] * s1
    w1q = (W1 * s1).reshape(FT, 128, KT, 128).transpose(3, 0, 2, 1).reshape(128, FT * C)
    # w2r[p, ft*768 + mt*128 + m] = W2[mt*128+m, ft*128+p]
    w2r = W2.T.reshape(FT, 128, C).transpose(1, 0, 2).reshape(128, FT * C)
    prep["w1q"] = np.ascontiguousarray(f8c(w1q))
    prep["w2r"] = bfc(w2r)
    prep["sc1"] = np.full((128, 1), 1.0 / (s1 * SX), np.float32)
    prep["b1"] = b1.reshape(FF, 1)
    prep["ones_sq"] = np.ones((128, 128), ml_dtypes.bfloat16)
    return prep


def kernel(**inputs):
    from concourse.bass_utils import run_bass_kernel_spmd

    _patch_compiler(ldw_opt=_BUILD_CACHE.get("ldw_opt", False))
    if "nc" not in _BUILD_CACHE:
        _BUILD_CACHE["nc"] = _build_program()
    nc = _BUILD_CACHE["nc"]

    prep = _host_prep(inputs)
    SHARDED = ("xTf", "xTb", "x8p", "xsq8")
    shared = {k: v for k, v in prep.items() if k not in SHARDED}
    in_maps = []
    for c in range(NCORES):
        im = dict(shared)
        for k in SHARDED:
            im[k] = np.ascontiguousarray(prep[k][c * BL:(c + 1) * BL])
        in_maps.append(im)

    res = run_bass_kernel_spmd(nc, in_maps, list(range(NCORES)),
                               **_BUILD_CACHE.get("run_kwargs", {}))
    _BUILD_CACHE["last_results"] = res
    outs = [res.results[c]["outT"].transpose(0, 2, 1) for c in range(NCORES)]
    return np.ascontiguousarray(np.concatenate(outs, 0).astype(np.float32))



# revision 69
# speedup vs baseline: 1.4292x; 1.0520x over previous
"""Trainium2 Bass kernel for nn_MixedAttentionModule (CvT-style mixed attention block).

Data-parallel over batch: 32 batches -> 8 cores x 4 batches. No collectives.
All layouts channel-major on device (activations [C, n]); host pre-transposes x
and post-transposes the output. LN/BN/bias affines are folded into adjacent
weights on the host. Depthwise 3x3 convs run on the tensor engine as 9
diagonal matmuls accumulating in PSUM. Attention computes scores^T = k q^T so
the softmax denominator is a ones-matmul and attn@v needs no transpose.
"""
import sys

sys.path.insert(0, "/opt/trn_rl_repo")

import numpy as np
import ml_dtypes

B, n, C, NH, HD, FF = 32, 1024, 768, 12, 64, 3072
Ht = Wt = 32
M = 256          # kv positions (16*16)
NCORES = 8
BL = B // NCORES  # batches per core
EPS = 1e-5
KT = C // 128     # 6 channel tiles
FT = FF // 128    # 24 ff tiles
NCH = 2           # n-chunks of 512
SX = 8.0          # fp8 scale on LN2 output (|ln| <= sqrt(C)=27.7, *8 = 222 < 240)
LN_SX = 2.0794415416798357   # ln(SX), folded into the rstd exp
SV = 16.0         # fp8 scale on v tokens (|v| ~ 0.8, *16 = 13 << 240)
NLN_SV = -2.772588722239781  # -ln(SV), folded into the sinv exp
CONV_DR = True    # fp8 DoubleRow conv: taps paired, pad/dw in fp8
SDW = 4.0         # fp8 scale on depthwise taps (|dw| ~ 0.4, *4 << 240)
# conv psum = (SX*SDW)*y; the elu chain emits 32*(elu(y)+1) (ln 32 in the exp)
LN_32 = 3.4657359027997265
F32 = None
BF16 = None

_BUILD_CACHE = {}


def _patch_compiler(ldw_opt=True):
    """Patch bass' walrus invocation: keep the standard pass list but allow
    toggling the LDWEIGHTS-dedup codegen optimization."""
    from pathlib import Path
    from concourse import bass_utils

    def patched(tmpdir, inp="bir.json", outp="file.neff", arch=None, *, dve_root=None):
        cmd = [
            bass_utils.get_walrus_driver(),
            "--pass",
            "birverifier,runtime_memory_reservation,lower_act,lower_dve,"
            "lower_ap_offset,codegen,neff_packager",
            "-i", inp,
            "--neff-output-filename", outp,
            "--enable-birsim=true",
            "--mem-mode=physical",
            "--policy=0",
            f"--enable-ldw-opt={'true' if ldw_opt else 'false'}",
            "--assign-static-dmas-to-sp=false",
            f"--dram-page-size={bass_utils.aot_getenv('NEURON_SCRATCHPAD_PAGE_SIZE', '256')}",
            "--enable-neff-debug-info=true",
            "--jobs", "8",
            *bass_utils.get_walrus_args(
                bass_utils.get_bir_arch(tmpdir, inp) if arch is None else arch,
                tmpdir, dve_root=dve_root,
            ),
        ]
        result = bass_utils.run_command(cmd, cwd=tmpdir)
        if result is not None:
            (Path(tmpdir) / "log.txt").write_text(result.stdout)
        return f"{tmpdir}/{outp}"

    bass_utils.bir_verify_and_optimise = patched


def _split_sync_waits(nc, max_waits=1):
    """walrus codegen in this environment allows at most one sync wait per
    instruction. Hoist excess waits onto standalone EventSemaphore carriers
    inserted just before, on the same engine (engines execute their stream
    in order, so this is equivalent)."""
    from concourse import mybir

    n_new = 0
    for f in nc.m.functions:
        for blk in f.blocks:
            out = []
            for inst in blk.instructions:
                si = getattr(inst, "sync_info", None)
                if si is not None:
                    waits = list(si.on_wait or [])
                    ups = list(si.on_update or [])
                    if len(waits) > max_waits:
                        extra = waits[: len(waits) - max_waits]
                        keep = waits[len(waits) - max_waits:]
                        for w in extra:
                            n_new += 1
                            out.append(mybir.InstEventSemaphore(
                                name=f"syncw-{n_new}-{inst.name}",
                                ins=[], outs=[],
                                engine=inst.engine,
                                sync_info=mybir.SyncInfo(on_wait=[w], on_update=[]),
                            ))
                        inst.sync_info = mybir.SyncInfo(on_wait=keep, on_update=ups)
                out.append(inst)
            blk.instructions = out
    return n_new


def _build_program():
    from concourse import bass, mybir, tile

    f32 = mybir.dt.float32
    bf16 = mybir.dt.bfloat16
    Alu = mybir.AluOpType
    Act = mybir.ActivationFunctionType
    DRM = mybir.MatmulPerfMode.DoubleRow

    f8 = mybir.dt.float8e4

    nc = bass.Bass("TRN2", target_bir_lowering=False, debug=False, num_devices=NCORES)

    # ---- DRAM I/O ----
    xTf = nc.dram_tensor("xTf", [BL, C, n], f32, kind="ExternalInput").ap()
    # fp8 x and x^2, packed [p, k*n+t] for DoubleRow LN1 stats
    x8p = nc.dram_tensor("x8p", [BL, 128, KT * n], f8, kind="ExternalInput").ap()
    xsq8 = nc.dram_tensor("xsq8", [BL, 128, KT * n], f8, kind="ExternalInput").ap()
    # fp8 projection weights packed for DoubleRow:
    #   wq8/wk8[p, mt*768 + kp*256 + i*128 + m] = W_eff[mt*128+m, (2kp+i)*128+p]*s
    #   wv8[p, k*768 + c] = Wv_eff[c, k*128+p]*s   (moving operand)
    wq8 = nc.dram_tensor("wq8", [128, KT * C], f8, kind="ExternalInput").ap()
    wk8 = nc.dram_tensor("wk8", [128, KT * C], f8, kind="ExternalInput").ap()
    wv8 = nc.dram_tensor("wv8", [128, KT * C], f8, kind="ExternalInput").ap()
    scq_d = nc.dram_tensor("scq", [128, 1], f32, kind="ExternalInput").ap()
    sck_d = nc.dram_tensor("sck", [128, 1], f32, kind="ExternalInput").ap()
    scv_d = nc.dram_tensor("scv", [128, 1], f32, kind="ExternalInput").ap()
    # fp8 W1 packed for DoubleRow; bf16 W2 packed per ft-tile:
    #   w1q[p, ft*768 + kp*256 + i*128 + f] = W1eff[ft*128+f, (2kp+i)*128+p] * s1
    #   w2r[p, ft*768 + mt*128 + m] = W2[mt*128+m, ft*128+p]
    w1q = nc.dram_tensor("w1q", [128, FT * C], f8, kind="ExternalInput").ap()
    w2r = nc.dram_tensor("w2r", [128, FT * C], bf16, kind="ExternalInput").ap()
    sc1_d = nc.dram_tensor("sc1", [128, 1], f32, kind="ExternalInput").ap()
    cdt = f8 if CONV_DR else bf16
    dq9 = nc.dram_tensor("dq9", [KT, 128, 9 * 128], cdt, kind="ExternalInput").ap()
    dk9 = nc.dram_tensor("dk9", [KT, 128, 9 * 128], cdt, kind="ExternalInput").ap()
    dv9 = nc.dram_tensor("dv9", [KT, 128, 9 * 128], cdt, kind="ExternalInput").ap()
    bq_d = nc.dram_tensor("bq", [C, 1], f32, kind="ExternalInput").ap()
    bk_d = nc.dram_tensor("bk", [C, 1], f32, kind="ExternalInput").ap()
    bva_d = nc.dram_tensor("bva", [C, 1], f32, kind="ExternalInput").ap()
    b1_d = nc.dram_tensor("b1", [FF, 1], f32, kind="ExternalInput").ap()
    ones_sq_d = nc.dram_tensor("ones_sq", [128, 128], bf16, kind="ExternalInput").ap()
    outT = nc.dram_tensor("outT", [BL, C, n], f32, kind="ExternalOutput").ap()

    with tile.TileContext(nc) as tc:
        with tc.tile_pool(name="P", bufs=1) as P:
            # ---- persistent SBUF (weights + per-batch activations) ----
            wq_sb = P.tile([128, KT * C], f8, name="wq8", tag="wq", bufs=1)
            wk_sb = P.tile([128, KT * C], f8, name="wk8", tag="wk", bufs=1)
            wv_sb = P.tile([128, KT * C], f8, name="wv8", tag="wv", bufs=1)
            bq6 = P.tile([128, KT], f32, name="bq6", tag="bq", bufs=1)
            bk6 = P.tile([128, KT], f32, name="bk6", tag="bk", bufs=1)
            bva6 = P.tile([128, KT], f32, name="bva6", tag="bva", bufs=1)
            b1_24 = P.tile([128, FT], f32, name="b1_24", tag="b1", bufs=1)
            ones_sq = P.tile([128, 128], bf16, name="onessq", tag="onessq", bufs=1)
            w1_sb = P.tile([128, FT * C], f8, name="w1q", tag="w1q", bufs=1)
            sc1_sb = P.tile([128, 1], f32, name="sc1", tag="sc1", bufs=1)
            scq_sb = P.tile([128, 1], f32, name="scq", tag="scq", bufs=1)
            sck_sb = P.tile([128, 1], f32, name="sck", tag="sck", bufs=1)
            scv_sb = P.tile([128, 1], f32, name="scv", tag="scv", bufs=1)

            # keep the sync queue free for batch 0's stats inputs; everything
            # needed later (biases, scales, projection weights) goes on gpsimd
            nc.gpsimd.dma_start(ones_sq[:], ones_sq_d[:, :])
            nc.gpsimd.dma_start(bq6[:], bq_d.rearrange("(t p) o -> p (t o)", p=128))
            nc.gpsimd.dma_start(bk6[:], bk_d.rearrange("(t p) o -> p (t o)", p=128))
            nc.gpsimd.dma_start(bva6[:], bva_d.rearrange("(t p) o -> p (t o)", p=128))
            nc.gpsimd.dma_start(b1_24[:], b1_d.rearrange("(t p) o -> p (t o)", p=128))
            nc.gpsimd.dma_start(sc1_sb[:], sc1_d[:, :])
            nc.gpsimd.dma_start(scq_sb[:], scq_d[:, :])
            nc.gpsimd.dma_start(sck_sb[:], sck_d[:, :])
            nc.gpsimd.dma_start(scv_sb[:], scv_d[:, :])
            nc.gpsimd.dma_start(wq_sb[:], wq8[:, :])
            nc.gpsimd.dma_start(wk_sb[:], wk8[:, :])
            nc.gpsimd.dma_start(wv_sb[:], wv8[:, :])

            pad = [P.tile([128, 34, 34], cdt, name=f"pad{k}", tag="pad", bufs=KT) for k in range(KT)]
            act8q = P.tile([128, KT * n], f8, name="a8q", tag="aq", bufs=1)
            act8k = P.tile([128, KT * M], f8, name="a8k", tag="ak", bufs=1)
            act8v = P.tile([128, KT * M], f8, name="a8v", tag="av", bufs=1)
            qT = [P.tile([128, n], bf16, name=f"qT{k}", tag="qT", bufs=KT) for k in range(KT)]
            kTt = [P.tile([128, M], bf16, name=f"kT{k}", tag="kT", bufs=KT) for k in range(KT)]
            vt8 = P.tile([128, 2, C], f8, name="vt8", tag="vt", bufs=1)
            ones8 = P.tile([128, 256], f8, name="ones8", tag="ones8", bufs=1)
            nc.vector.memset(ones8[:], 1.0)
            # half-zeroed ones stationaries: accumulate both heads' kv-sums into
            # disjoint partition halves of one PSUM tile
            ones_hf = [P.tile([128, 256], f8, name=f"oneshf{hh}", tag="oneshf", bufs=2)
                       for hh in range(2)]
            for hh in range(2):
                nc.vector.memset(ones_hf[hh][:], 0.0)
                nc.vector.memset(ones_hf[hh][:, hh * 64:hh * 64 + 64], 1.0)
                nc.vector.memset(ones_hf[hh][:, 128 + hh * 64:128 + hh * 64 + 64], 1.0)
            OT = [P.tile([128, n], bf16, name=f"OT{k}", tag="OT", bufs=KT) for k in range(KT)]
            x2b = [P.tile([128, n], bf16, name=f"x2{k}", tag="x2", bufs=KT) for k in range(KT)]
            # LN2 output: fp8, all 6 channel tiles in one buffer so DoubleRow can
            # pair adjacent k-tiles along the free dim (stride n between planes)
            xl8 = P.tile([128, KT * n], f8, name="xl8", tag="xl8", bufs=1)

            # zero the padded conv buffers once (interiors are overwritten per batch;
            # the one-element borders must stay zero)
            for k in range(KT):
                nc.vector.memset(pad[k][:], 0.0)

            # constant APs for float biases of activation ops
            czero = P.tile([128, 1], f32, name="czero", tag="cz", bufs=2)
            nc.vector.memset(czero[:], 0.0)
            nc.const_aps.aps[(f32, 0.0)] = czero[:]
            ceps = P.tile([128, 1], f32, name="ceps", tag="cz", bufs=2)
            nc.vector.memset(ceps[:], EPS)
            nc.const_aps.aps[(f32, EPS)] = ceps[:]
            cln8 = P.tile([128, 1], f32, name="cln8", tag="cln8", bufs=1)
            nc.vector.memset(cln8[:], LN_SX)
            nc.const_aps.aps[(f32, LN_SX)] = cln8[:]
            cnv = P.tile([128, 1], f32, name="cnv", tag="cnv", bufs=1)
            nc.vector.memset(cnv[:], NLN_SV)
            nc.const_aps.aps[(f32, NLN_SV)] = cnv[:]
            cl32 = P.tile([128, 1], f32, name="cl32", tag="cl32", bufs=1)
            nc.vector.memset(cl32[:], LN_32)
            nc.const_aps.aps[(f32, LN_32)] = cl32[:]

            def ln_finish(ps_mean, ps_msq, src_of, dst_write, label, bno, rstd_bias):
                mbcs, rbcs = [], []
                for ch in range(NCH):
                    mbc = P.tile([128, 512], f32, name=f"mbc{label}{bno}_{ch}", tag="mbc", bufs=2)
                    rbc = P.tile([128, 512], f32, name=f"rbc{label}{bno}_{ch}", tag="rbc", bufs=2)
                    nc.vector.tensor_scalar_mul(mbc[:], ps_mean[ch][:], 1.0 / C)
                    # rstd = 1/sqrt((msq/C) - mean^2 + eps)
                    nc.vector.tensor_mul(rbc[:], mbc[:], mbc[:])
                    nc.vector.scalar_tensor_tensor(rbc[:], ps_msq[ch][:], 1.0 / C,
                                                   rbc[:], Alu.mult, Alu.subtract)
                    # rstd = exp(-0.5*ln(var+eps)) on ACT (keeps DVE free;
                    # table accuracy ~1e-4 rel, far below bf16 noise);
                    # rstd_bias multiplies rstd by exp(rstd_bias) for free
                    nc.scalar.activation(rbc[:], rbc[:], Act.Ln, bias=EPS)
                    nc.scalar.activation(rbc[:], rbc[:], Act.Exp, scale=-0.5,
                                         bias=rstd_bias)
                    mbcs.append(mbc); rbcs.append(rbc)
                # k-major apply order so the consumer (conv k=0 / FFN kp=0)
                # unblocks after two writes instead of seven
                for k in range(KT):
                    for ch in range(NCH):
                        sl = slice(ch * 512, (ch + 1) * 512)
                        dst_write(k, ch, src_of(k, sl), mbcs[ch], rbcs[ch])

            ones8v = ones8[:].rearrange("p (i f) -> p i f", i=2)

            def ln_stats_apply(src_tiles, dst_write, label, bno, rstd_bias=0.0):
                """src_tiles: 6 bf16 [128, n] channel-major tiles.
                Stats matmuls use a ones[128,128] stationary so the channel-sums
                arrive pre-broadcast across all 128 partitions; all row math is
                then full-width DVE work and no PE broadcast is needed."""
                with tc.tile_pool(name=f"ps_ln_{label}{bno}", bufs=1, space="PSUM") as psp:
                    ps_mean = [psp.tile([128, 512], f32, name=f"psm{label}{bno}_{c}", tag="mm", bufs=4) for c in range(NCH)]
                    ps_msq = [psp.tile([128, 512], f32, name=f"psq{label}{bno}_{c}", tag="mm", bufs=4) for c in range(NCH)]
                    # squares on ACT (bf16 out), then ones-matmul stats; groups are
                    # interleaved across banks so sq tiles can double-buffer
                    for k in range(KT):
                        sqt = P.tile([128, n], bf16, name=f"sq{label}{bno}_{k}", tag="sq", bufs=2)
                        nc.scalar.activation(sqt[:], src_tiles[k][:], Act.Square)
                        for ch in range(NCH):
                            sl = slice(ch * 512, (ch + 1) * 512)
                            nc.tensor.matmul(ps_mean[ch][:], ones_sq[:], src_tiles[k][:, sl],
                                             start=(k == 0), stop=(k == KT - 1))
                            nc.tensor.matmul(ps_msq[ch][:], ones_sq[:], sqt[:, sl],
                                             start=(k == 0), stop=(k == KT - 1))
                    ln_finish(ps_mean, ps_msq, lambda k, sl: src_tiles[k][:, sl],
                              dst_write, label, bno, rstd_bias)

            def ln1_stats_apply(dst_write, bno, xb8_t, xsq8_t):
                """LN1 stats from host-precomputed fp8 x and x^2 via DoubleRow
                ones-matmuls (half the PE passes, no device squares)."""
                with tc.tile_pool(name=f"ps_ln_a{bno}", bufs=1, space="PSUM") as psp:
                    ps_mean = [psp.tile([128, 512], f32, name=f"psma{bno}_{c}", tag="mm", bufs=4) for c in range(NCH)]
                    ps_msq = [psp.tile([128, 512], f32, name=f"psqa{bno}_{c}", tag="mm", bufs=4) for c in range(NCH)]
                    for kp in range(KT // 2):
                        for ch in range(NCH):
                            sl = slice(ch * 512, (ch + 1) * 512)
                            xap = xb8_t[:, 2 * kp * n:(2 * kp + 2) * n] \
                                .rearrange("p (i t) -> p i t", i=2)[:, :, sl]
                            sap = xsq8_t[:, 2 * kp * n:(2 * kp + 2) * n] \
                                .rearrange("p (i t) -> p i t", i=2)[:, :, sl]
                            nc.tensor.matmul(ps_mean[ch][:], ones8v, xap,
                                             start=(kp == 0), stop=(kp == KT // 2 - 1),
                                             perf_mode=DRM)
                            nc.tensor.matmul(ps_msq[ch][:], ones8v, sap,
                                             start=(kp == 0), stop=(kp == KT // 2 - 1),
                                             perf_mode=DRM)
                    ln_finish(ps_mean, ps_msq, lambda k, sl: xb8_t[:, k * n:(k + 1) * n][:, sl],
                              dst_write, "a", bno, LN_SX if CONV_DR else 0.0)

            # per-batch fp8 stats inputs, double-buffered and prefetched during
            # the previous batch's attention phase
            xstats = {}

            def fetch_x(bno):
                if bno >= BL:
                    return
                t1 = P.tile([128, KT * n], f8, name=f"xb8_{bno}", tag="xb8", bufs=2)
                t2 = P.tile([128, KT * n], f8, name=f"xsq8_{bno}", tag="xsq8", bufs=2)
                nc.sync.dma_start(t1[:], x8p[bno, :, :])
                nc.sync.dma_start(t2[:], xsq8[bno, :, :])
                xstats[bno] = (t1, t2)

            fetch_x(0)
            for b in range(BL):
                # ---------------- LN1 + conv + projections ----------------
                xb8_t, xsq8_t = xstats.pop(b)

                def ln1_write(k, ch, src, mbc, rbc):
                    tmp = P.tile([128, 512], f32, name=f"t1_{b}_{k}_{ch}", tag="tmp", bufs=3)
                    nc.vector.tensor_sub(tmp[:], src, mbc[:])
                    # write normalized values into padded interior rows (fp8,
                    # prescaled by SX via the rstd bias when CONV_DR)
                    r0 = 1 + 16 * ch
                    dst = pad[k][:, r0:r0 + 16, 1:33]
                    nc.vector.tensor_mul(dst, tmp[:].rearrange("p (a c) -> p a c", a=16), rbc[:].rearrange("p (a c) -> p a c", a=16))
                    return

                ln1_stats_apply(ln1_write, b, xb8_t, xsq8_t)

                # conv: fp8 DoubleRow with taps paired (4 pairs + 1 single per
                # 3x3 kernel); psum = SX*SDW*y, the elu chain emits 32*(elu+1)
                with tc.tile_pool(name=f"ps_conv{b}", bufs=1, space="PSUM") as cvp:
                    def elu_chain(ps_ap, dst_ap, width):
                        tmin = P.tile([128, width], f32, name=f"tm{b}", tag="tmpe", bufs=3)
                        et = P.tile([128, width], bf16, name=f"ee{b}", tag="ee", bufs=3)
                        nc.vector.tensor_scalar_min(tmin[:], ps_ap, 0.0)
                        if CONV_DR:
                            # 32*e^{min(y,0)} with y = psum/32
                            nc.scalar.activation(et[:], tmin[:], Act.Exp,
                                                 scale=1.0 / (SX * SDW), bias=LN_32)
                        else:
                            nc.scalar.activation(et[:], tmin[:], Act.Exp)
                        # 32*(elu+1) = relu(psum) + 32*exp(min(y,0)); the scale
                        # and the -1 are folded into the projection weights/biases
                        nc.vector.scalar_tensor_tensor(dst_ap, ps_ap, 0.0, et[:], Alu.max, Alu.add)

                    def conv_pair_ap(k, base_r, base_c, pr, rows, rstride, cstep):
                        """moving AP [128, 2, rows, 32/16]: tap pair (2pr, 2pr+1)
                        windows of the padded image (overlapping strides)."""
                        t0, t1 = 2 * pr, 2 * pr + 1
                        o0 = (t0 // 3 + base_r) * 34 + (t0 % 3) + base_c
                        o1 = (t1 // 3 + base_r) * 34 + (t1 % 3) + base_c
                        a = pad[k][:, 0:rows, 0:32:cstep].unsqueeze(1)
                        V = type(a.ap)
                        pdim = tuple(a.ap[0])
                        a.ap = V([pdim, (o1 - o0, 2), (34 * rstride, rows), (cstep, 32 // cstep)])
                        a.offset = a.offset + o0
                        return a

                    def conv_single_ap(k, base_r, base_c, tap, rows, rstride, cstep):
                        dy, dx = tap // 3, tap % 3
                        if rstride == 1:
                            return pad[k][:, base_r + dy:base_r + dy + rows, dx:dx + 32]
                        return pad[k][:, dy:dy + 32:2, dx:dx + 32:2]

                    def conv_mms(k, dt8, out_ps, base_r, rows, rstride, cstep):
                        for pr in range(4):
                            wap = dt8[:, pr * 256:(pr + 1) * 256].rearrange("p (i c) -> p i c", i=2)
                            nc.tensor.matmul(out_ps, wap,
                                             conv_pair_ap(k, base_r, 0, pr, rows, rstride, cstep),
                                             start=(pr == 0), stop=False, perf_mode=DRM)
                        nc.tensor.matmul(out_ps, dt8[:, 1024:1152],
                                         conv_single_ap(k, base_r, 0, 8, rows, rstride, cstep),
                                         start=False, stop=True)

                    for k in range(KT):
                        dqt = P.tile([128, 9 * 128], cdt, name=f"dq{b}_{k}", tag="dq", bufs=2)
                        nc.gpsimd.dma_start(dqt[:], dq9[k, :, :])
                        pq = [cvp.tile([128, 512], f32, name=f"pcq{b}_{k}_{c}", tag="mm", bufs=4) for c in range(NCH)]
                        for ch in range(NCH):
                            conv_mms(k, dqt, pq[ch][:], 16 * ch, 16, 1, 1)
                        for ch in range(NCH):
                            elu_chain(pq[ch][:], act8q[:, k * n + ch * 512:k * n + (ch + 1) * 512], 512)
                    for k in range(KT):
                        dkt = P.tile([128, 9 * 128], cdt, name=f"dk{b}_{k}", tag="dkv", bufs=2)
                        dvt = P.tile([128, 9 * 128], cdt, name=f"dv{b}_{k}", tag="dkv", bufs=2)
                        nc.sync.dma_start(dkt[:], dk9[k, :, :])
                        nc.sync.dma_start(dvt[:], dv9[k, :, :])
                        pk = cvp.tile([128, M], f32, name=f"pck{b}_{k}", tag="mm", bufs=4)
                        pv = cvp.tile([128, M], f32, name=f"pcv{b}_{k}", tag="mm", bufs=4)
                        conv_mms(k, dkt, pk[:], 0, 16, 2, 2)
                        conv_mms(k, dvt, pv[:], 0, 16, 2, 2)
                        elu_chain(pk[:], act8k[:, k * M:(k + 1) * M], M)
                        elu_chain(pv[:], act8v[:, k * M:(k + 1) * M], M)

                    # projections -- fp8 DoubleRow over contraction-tile pairs;
                    # dequant scale + bias applied in one DVE tensor_scalar
                    def a8pair(act8, width, kp, sl2):
                        return act8[:, 2 * kp * width:(2 * kp + 2) * width] \
                            .rearrange("p (i t) -> p i t", i=2)[:, :, sl2]

                    for mt in range(KT):
                        psq = [cvp.tile([128, 512], f32, name=f"pq{b}_{mt}_{c}", tag="mm", bufs=4)
                               for c in range(NCH)]
                        for kp in range(KT // 2):
                            wap = wq_sb[:, mt * C + kp * 256:mt * C + (kp + 1) * 256] \
                                .rearrange("p (i f) -> p i f", i=2)
                            for ch in range(NCH):
                                nc.tensor.matmul(psq[ch][:], wap,
                                                 a8pair(act8q, n, kp, slice(ch * 512, (ch + 1) * 512)),
                                                 start=(kp == 0), stop=(kp == KT // 2 - 1),
                                                 perf_mode=DRM)
                        for ch in range(NCH):
                            nc.vector.tensor_scalar(qT[mt][:, ch * 512:(ch + 1) * 512], psq[ch][:],
                                                    scq_sb[:], bq6[:, mt:mt + 1],
                                                    Alu.mult, Alu.add)
                    for mt in range(KT):
                        ps = cvp.tile([128, M], f32, name=f"pk{b}_{mt}", tag="mm", bufs=4)
                        for kp in range(KT // 2):
                            wap = wk_sb[:, mt * C + kp * 256:mt * C + (kp + 1) * 256] \
                                .rearrange("p (i f) -> p i f", i=2)
                            nc.tensor.matmul(ps[:], wap, a8pair(act8k, M, kp, slice(0, M)),
                                             start=(kp == 0), stop=(kp == KT // 2 - 1),
                                             perf_mode=DRM)
                        nc.vector.tensor_scalar(kTt[mt][:, :], ps[:],
                                                sck_sb[:], bk6[:, mt:mt + 1],
                                                Alu.mult, Alu.add)
                    for mt2 in range(2):
                        psv = [cvp.tile([128, w], f32, name=f"pv{b}_{mt2}_{c}", tag="mm", bufs=4)
                               for c, w in [(0, 512), (1, 256)]]
                        for kp in range(KT // 2):
                            aap = a8pair(act8v, M, kp, slice(mt2 * 128, (mt2 + 1) * 128))
                            for ch, w in [(0, 512), (1, 256)]:
                                nc.tensor.matmul(psv[ch][:], aap,
                                                 wv_sb[:, 2 * kp * C:(2 * kp + 2) * C]
                                                 .rearrange("p (i c) -> p i c", i=2)[:, :, ch * 512:ch * 512 + w],
                                                 start=(kp == 0), stop=(kp == KT // 2 - 1),
                                                 perf_mode=DRM)
                        for ch, w in [(0, 512), (1, 256)]:
                            # v tokens in fp8, prescaled by SV/s_wv (folded out via sinv)
                            nc.vector.tensor_scalar(
                                vt8[:, mt2:mt2 + 1, ch * 512:ch * 512 + w], psv[ch][:],
                                scv_sb[:], None, Alu.mult)

                if b == 0:
                    # one-time fp8 W1 load; queued here so batch 0's conv
                    # weights (same gpsimd queue) aren't delayed behind it
                    for half in range(4):
                        slh = slice(half * (FT * C // 4), (half + 1) * (FT * C // 4))
                        nc.gpsimd.dma_start(w1_sb[:, slh], w1q[:, slh])

                # prefetch next batch's stats inputs while the sync queue is idle
                fetch_x(b + 1)

                # ---------------- attention ----------------
                # software-pipelined over head pairs: scores(j+1) are emitted
                # before sum/AV(j) so the PE streams while ACT runs the exps
                with tc.tile_pool(name=f"ps_at{b}", bufs=1, space="PSUM") as atp:
                    def att_scores(j):
                        # exp(scores) in fp8, kv tiles stacked for DoubleRow;
                        # alternate the two heads' row-halves so the PE streams
                        # both halves concurrently
                        ET2 = [P.tile([128, 2, n], f8, name=f"ET{b}_{j}_{hh}", tag="ET", bufs=4)
                               for hh in range(2)]
                        for mt in range(2):
                            for ch in range(NCH):
                                for hh in range(2):
                                    bp = 64 * hh
                                    ps = atp.tile([128, 512], f32, name=f"pss{b}_{j}_{hh}_{mt}_{ch}", tag="mm", bufs=4)
                                    nc.tensor.matmul(ps[:],
                                                     kTt[j][bp:bp + 64, mt * 128:(mt + 1) * 128],
                                                     qT[j][bp:bp + 64, ch * 512:(ch + 1) * 512],
                                                     tile_position=(bp, 0))
                                    nc.scalar.activation(ET2[hh][:, mt:mt + 1, ch * 512:(ch + 1) * 512],
                                                         ps[:], Act.Exp, scale=0.125)
                        return ET2

                    def att_finish(j, ET2):
                        # kv-sums of both heads accumulate into disjoint partition
                        # halves of one PSUM tile (half-zeroed ones stationaries),
                        # so Ln/Exp run once per chunk at full width
                        sinv = [P.tile([128, 512], f32, name=f"si{b}_{j}_{c}", tag="sinv", bufs=4)
                                for c in range(NCH)]
                        for ch in range(NCH):
                            sum_ps = atp.tile([128, 512], f32, name=f"psum{b}_{j}_{ch}", tag="bc", bufs=2)
                            for hh in range(2):
                                nc.tensor.matmul(sum_ps[:],
                                                 ones_hf[hh][:].rearrange("p (i f) -> p i f", i=2),
                                                 ET2[hh][:, :, ch * 512:(ch + 1) * 512],
                                                 start=(hh == 0), stop=(hh == 1),
                                                 perf_mode=DRM)
                            # 1/(s*SV) = exp(-ln(s) - ln SV) on ACT
                            nc.scalar.activation(sinv[ch][:], sum_ps[:], Act.Ln)
                            nc.scalar.activation(sinv[ch][:], sinv[ch][:], Act.Exp,
                                                 scale=-1.0, bias=NLN_SV)
                        po = [atp.tile([128, 512], f32, name=f"po{b}_{j}_{c}", tag="o", bufs=2)
                              for c in range(NCH)]
                        for mt in range(2):
                            for ch in range(NCH):
                                for hh in range(2):
                                    bp = 64 * hh
                                    h = 2 * j + hh
                                    nc.tensor.matmul(po[ch][bp:bp + 64, :],
                                                     vt8[:, mt:mt + 1, h * 64:(h + 1) * 64],
                                                     ET2[hh][:, mt:mt + 1, ch * 512:(ch + 1) * 512],
                                                     start=(mt == 0), stop=(mt == 1),
                                                     tile_position=(0, bp))
                        for ch in range(NCH):
                            sl = slice(ch * 512, (ch + 1) * 512)
                            nc.vector.tensor_mul(OT[j][:, sl], po[ch][:], sinv[ch][:])

                    ET_prev = att_scores(0)
                    for j in range(1, NH // 2):
                        ET_cur = att_scores(j)
                        att_finish(j - 1, ET_prev)
                        ET_prev = ET_cur
                    att_finish(NH // 2 - 1, ET_prev)

                # ---------------- residual + LN2 ----------------
                for k in range(KT):
                    for ch in range(NCH):
                        sl = slice(ch * 512, (ch + 1) * 512)
                        xf = P.tile([128, 512], f32, name=f"xf{b}_{k}_{ch}", tag="xf", bufs=3)
                        nc.sync.dma_start(xf[:], xTf[b, k * 128:(k + 1) * 128, sl])
                        nc.vector.scalar_tensor_tensor(x2b[k][:, sl], OT[k][:, sl], bva6[:, k:k + 1], xf[:],
                                                       Alu.add, Alu.add)

                def ln2_write(k, ch, src, mbc, rbc):
                    tmp = P.tile([128, 512], f32, name=f"t2_{b}_{k}_{ch}", tag="tmp", bufs=3)
                    nc.vector.tensor_sub(tmp[:], src, mbc[:])
                    # rbc carries exp(ln 8) = SX, so this writes xn*8 in fp8e4
                    nc.vector.tensor_mul(xl8[:, k * n + ch * 512:k * n + (ch + 1) * 512],
                                         tmp[:], rbc[:])

                ln_stats_apply(x2b, ln2_write, "c", b, rstd_bias=LN_SX)

                # ---------------- FFN (fp8 DoubleRow h1, bf16 h2) + residual ----------------
                # software-pipelined: h1(ft+1) is emitted before h2(ft) so the
                # PE streams through the gelu latency
                with tc.tile_pool(name=f"ps_ffn{b}", bufs=1, space="PSUM") as ffp:
                    for ch in range(NCH):
                        sl = slice(ch * 512, (ch + 1) * 512)
                        ph2 = [ffp.tile([128, 512], f32, name=f"ph2_{b}_{ch}_{mt}", tag="h2", bufs=6)
                               for mt in range(KT)]
                        ph1s, w2bs = {}, {}

                        def fetch_w2(ft):
                            if ft >= FT:
                                return
                            w2b = P.tile([128, C], bf16, name=f"w2_{b}_{ch}_{ft}", tag="w2", bufs=4)
                            nc.gpsimd.dma_start(w2b[:], w2r[:, ft * C:(ft + 1) * C])
                            w2bs[ft] = w2b

                        def emit_h1(ft):
                            ph1 = ffp.tile([128, 512], f32, name=f"ph1_{b}_{ch}_{ft}", tag="h1", bufs=2)
                            for kp in range(KT // 2):
                                w1ap = w1_sb[:, ft * C + kp * 256: ft * C + (kp + 1) * 256] \
                                    .rearrange("p (i f) -> p i f", i=2)
                                xap = xl8[:, 2 * kp * n:(2 * kp + 2) * n] \
                                    .rearrange("p (i t) -> p i t", i=2)[:, :, sl]
                                nc.tensor.matmul(ph1[:], w1ap, xap,
                                                 start=(kp == 0), stop=(kp == KT // 2 - 1),
                                                 perf_mode=DRM)
                            ph1s[ft] = ph1

                        fetch_w2(0)
                        fetch_w2(1)
                        emit_h1(0)
                        for ft in range(FT):
                            fetch_w2(ft + 2)
                            if ft + 1 < FT:
                                emit_h1(ft + 1)
                            gt = P.tile([128, 512], bf16, name=f"g_{b}_{ch}_{ft}", tag="g", bufs=3)
                            nc.scalar.activation(gt[:], ph1s.pop(ft)[:], Act.Gelu,
                                                 bias=b1_24[:, ft:ft + 1], scale=sc1_sb[:])
                            w2b = w2bs.pop(ft)
                            for mt in range(KT):
                                nc.tensor.matmul(ph2[mt][:],
                                                 w2b[:, mt * 128:(mt + 1) * 128],
                                                 gt[:],
                                                 start=(ft == 0), stop=(ft == FT - 1))
                        for mt in range(KT):
                            xf2 = P.tile([128, 512], f32, name=f"xf2_{b}_{ch}_{mt}", tag="xf", bufs=3)
                            nc.sync.dma_start(xf2[:], xTf[b, mt * 128:(mt + 1) * 128, sl])
                            ub = P.tile([128, 512], f32, name=f"u_{b}_{ch}_{mt}", tag="tmp", bufs=3)
                            nc.vector.scalar_tensor_tensor(ub[:], OT[mt][:, sl], bva6[:, mt:mt + 1], xf2[:],
                                                           Alu.add, Alu.add)
                            ob = P.tile([128, 512], f32, name=f"o_{b}_{ch}_{mt}", tag="ob", bufs=3)
                            nc.vector.tensor_add(ob[:], ub[:], ph2[mt][:])
                            nc.gpsimd.dma_start(outT[b, mt * 128:(mt + 1) * 128, sl], ob[:])
    n_hoisted = _split_sync_waits(nc)
    print(f"_split_sync_waits: hoisted waits onto {n_hoisted} carrier instructions")
    return nc


def _host_prep(inputs):
    """Fold LN/BN affines into weights; build packed bf16 arrays."""
    f = lambda k: np.asarray(inputs[k], np.float32)
    bfc = lambda a: np.ascontiguousarray(a.astype(ml_dtypes.bfloat16))
    x = f("x")                         # (B, n, C)
    ln1_g, ln1_b = f("ln1_g"), f("ln1_b")
    ln2_g, ln2_b = f("ln2_g"), f("ln2_b")

    f8c = lambda a: np.clip(a, -240.0, 240.0).astype(ml_dtypes.float8_e4m3)
    prep = {}
    xT = np.ascontiguousarray(x.transpose(0, 2, 1))   # (B, C, n)
    prep["xTf"] = xT
    # fp8 x and x^2 packed [b, p, k*n + t] for DoubleRow LN1 stats
    xp = xT.reshape(B, KT, 128, Ht * Wt).transpose(0, 2, 1, 3).reshape(B, 128, KT * Ht * Wt)
    prep["x8p"] = np.ascontiguousarray(f8c(xp))
    prep["xsq8"] = np.ascontiguousarray(f8c(xp * xp))

    diag9 = {}
    badj = {}
    for nm in ["q", "k", "v"]:
        w = f(f"dw_w_{nm}")[:, 0]                     # (C,3,3)
        w_eff = w * ln1_g[:, None, None]
        cb = f(f"dw_b_{nm}") + ln1_b * w.sum((1, 2))  # exact only if ln1_b == 0 (boundary)
        assert np.abs(cb).max() < 1e-30, "nonzero conv bias not implemented on device"
        sc = f(f"bn_g_{nm}") / np.sqrt(f(f"bn_v_{nm}") + EPS)
        sh = f(f"bn_b_{nm}") - f(f"bn_m_{nm}") * sc
        W = f(f"W_{nm}")
        W_eff = W * sc[None, :]
        # with CONV_DR the device act is 32*(elu+1); fold the /32 into W here
        CA = SX * SDW if CONV_DR else 1.0
        s_w = 2.0 ** np.floor(np.log2(224.0 * CA / max(np.abs(W_eff).max(), 1e-30)))
        Wq8 = f8c(W_eff * (s_w / CA))
        # the device multiplies with the fp8 weights, so the elu+1 "-1" fold
        # must subtract the row sums of the QUANTIZED weights or a constant
        # per-channel offset (Wq-W).sum(1) leaks into the output
        W_deq = Wq8.astype(np.float32) * (CA / s_w)
        b_eff = f(f"b_{nm}") + W @ sh - W_deq.sum(1)
        # pack tap matrices: 4 DoubleRow pairs + 1 single when CONV_DR
        # (diag pairs [pr, i, c]), else 9 diagonal taps
        d = np.zeros((KT, 128, 9 * 128), np.float32)
        wpack = w_eff * SDW if CONV_DR else w_eff
        for kt in range(KT):
            ww = wpack[kt * 128:(kt + 1) * 128]       # (128,3,3)
            for tap in range(9):
                dy, dx = tap // 3, tap % 3
                d[kt, np.arange(128), tap * 128 + np.arange(128)] = ww[:, dy, dx]
        diag9[nm] = f8c(d) if CONV_DR else bfc(d)
        badj[nm] = b_eff
        if nm == "v":
            # moving operand: wv8[p, k*768 + c] = (W_eff/CA).T[k*128+p, c] * s
            wv = Wq8.T.reshape(KT, 128, C).transpose(1, 0, 2).reshape(128, KT * C)
            prep["wv8"] = np.ascontiguousarray(wv)
            prep["scv"] = np.full((128, 1), SV / s_w, np.float32)
        else:
            # stationary: w8[p, mt*768 + kp*256 + i*128 + m] = Wq8[mt*128+m, (2kp+i)*128+p]
            wq = Wq8.reshape(KT, 128, KT, 128).transpose(3, 0, 2, 1).reshape(128, KT * C)
            prep[f"w{nm}8"] = np.ascontiguousarray(wq)
            prep[f"sc{nm}"] = np.full((128, 1), 1.0 / s_w, np.float32)
    prep["dq9"], prep["dk9"], prep["dv9"] = diag9["q"], diag9["k"], diag9["v"]
    prep["bq"] = badj["q"].reshape(C, 1)
    prep["bk"] = badj["k"].reshape(C, 1)
    prep["bva"] = badj["v"].reshape(C, 1)

    W1 = f("W1") * ln2_g[None, :]                     # (FF, C)
    b1 = f("b1") + f("W1") @ ln2_b
    W2 = f("W2")                                      # (C, FF)
    assert np.abs(f("b2")).max() < 1e-30, "nonzero b2 not implemented on device"
    # fp8e4 (TRN: max +-240) DoubleRow packing, power-of-2 per-tensor scale
    s1 = 2.0 ** np.floor(np.log2(224.0 / max(np.abs(W1).max(), 1e-30)))
    # w1q[p, ft*768 + kp*256 + i*128 + f] = W1[ft*128+f, (2kp+i)*128+p] * s1
    w1q = (W1 * s1).reshape(FT, 128, KT, 128).transpose(3, 0, 2, 1).reshape(128, FT * C)
    # w2r[p, ft*768 + mt*128 + m] = W2[mt*128+m, ft*128+p]
    w2r = W2.T.reshape(FT, 128, C).transpose(1, 0, 2).reshape(128, FT * C)
    prep["w1q"] = np.ascontiguousarray(f8c(w1q))
    prep["w2r"] = bfc(w2r)
    prep["sc1"] = np.full((128, 1), 1.0 / (s1 * SX), np.float32)
    prep["b1"] = b1.reshape(FF, 1)
    prep["ones_sq"] = np.ones((128, 128), ml_dtypes.bfloat16)
    return prep


def kernel(**inputs):
    from concourse.bass_utils import run_bass_kernel_spmd

    _patch_compiler(ldw_opt=_BUILD_CACHE.get("ldw_opt", False))
    if "nc" not in _BUILD_CACHE:
        _BUILD_CACHE["nc"] = _build_program()
    nc = _BUILD_CACHE["nc"]

    prep = _host_prep(inputs)
    SHARDED = ("xTf", "x8p", "xsq8")
    shared = {k: v for k, v in prep.items() if k not in SHARDED}
    in_maps = []
    for c in range(NCORES):
        im = dict(shared)
        for k in SHARDED:
            im[k] = np.ascontiguousarray(prep[k][c * BL:(c + 1) * BL])
        in_maps.append(im)

    res = run_bass_kernel_spmd(nc, in_maps, list(range(NCORES)),
                               **_BUILD_CACHE.get("run_kwargs", {}))
    _BUILD_CACHE["last_results"] = res
    outs = [res.results[c]["outT"].transpose(0, 2, 1) for c in range(NCORES)]
    return np.ascontiguousarray(np.concatenate(outs, 0).astype(np.float32))



# revision 80
# speedup vs baseline: 1.4993x; 1.0490x over previous
"""Trainium2 Bass kernel for nn_MixedAttentionModule (CvT-style mixed attention block).

Data-parallel over batch: 32 batches -> 8 cores x 4 batches. No collectives.
All layouts channel-major on device (activations [C, n]); host pre-transposes x
and post-transposes the output. LN/BN/bias affines are folded into adjacent
weights on the host. Depthwise 3x3 convs run on the tensor engine as 9
diagonal matmuls accumulating in PSUM. Attention computes scores^T = k q^T so
the softmax denominator is a ones-matmul and attn@v needs no transpose.
"""
import sys

sys.path.insert(0, "/opt/trn_rl_repo")

import numpy as np
import ml_dtypes

B, n, C, NH, HD, FF = 32, 1024, 768, 12, 64, 3072
Ht = Wt = 32
M = 256          # kv positions (16*16)
NCORES = 8
BL = B // NCORES  # batches per core
EPS = 1e-5
KT = C // 128     # 6 channel tiles
FT = FF // 128    # 24 ff tiles
NCH = 2           # n-chunks of 512
SX = 8.0          # fp8 scale on LN2 output (|ln| <= sqrt(C)=27.7, *8 = 222 < 240)
LN_SX = 2.0794415416798357   # ln(SX), folded into the rstd exp
SV = 16.0         # fp8 scale on v tokens (|v| ~ 0.8, *16 = 13 << 240)
NLN_SV = -2.772588722239781  # -ln(SV), folded into the sinv exp
CONV_DR = True    # fp8 DoubleRow conv: taps paired, pad/dw in fp8
SDW = 4.0         # fp8 scale on depthwise taps (|dw| ~ 0.4, *4 << 240)
# conv psum = (SX*SDW)*y; the elu chain emits 32*(elu(y)+1) (ln 32 in the exp)
LN_32 = 3.4657359027997265
F32 = None
BF16 = None

_BUILD_CACHE = {}


def _patch_compiler(ldw_opt=True):
    """Patch bass' walrus invocation: keep the standard pass list but allow
    toggling the LDWEIGHTS-dedup codegen optimization."""
    from pathlib import Path
    from concourse import bass_utils

    def patched(tmpdir, inp="bir.json", outp="file.neff", arch=None, *, dve_root=None):
        cmd = [
            bass_utils.get_walrus_driver(),
            "--pass",
            "birverifier,runtime_memory_reservation,lower_act,lower_dve,"
            "lower_ap_offset,codegen,neff_packager",
            "-i", inp,
            "--neff-output-filename", outp,
            "--enable-birsim=true",
            "--mem-mode=physical",
            "--policy=0",
            f"--enable-ldw-opt={'true' if ldw_opt else 'false'}",
            "--assign-static-dmas-to-sp=false",
            f"--dram-page-size={bass_utils.aot_getenv('NEURON_SCRATCHPAD_PAGE_SIZE', '256')}",
            "--enable-neff-debug-info=true",
            "--jobs", "8",
            *bass_utils.get_walrus_args(
                bass_utils.get_bir_arch(tmpdir, inp) if arch is None else arch,
                tmpdir, dve_root=dve_root,
            ),
        ]
        result = bass_utils.run_command(cmd, cwd=tmpdir)
        if result is not None:
            (Path(tmpdir) / "log.txt").write_text(result.stdout)
        return f"{tmpdir}/{outp}"

    bass_utils.bir_verify_and_optimise = patched


def _split_sync_waits(nc, max_waits=1):
    """walrus codegen in this environment allows at most one sync wait per
    instruction. Hoist excess waits onto standalone EventSemaphore carriers
    inserted just before, on the same engine (engines execute their stream
    in order, so this is equivalent)."""
    from concourse import mybir

    n_new = 0
    for f in nc.m.functions:
        for blk in f.blocks:
            out = []
            for inst in blk.instructions:
                si = getattr(inst, "sync_info", None)
                if si is not None:
                    waits = list(si.on_wait or [])
                    ups = list(si.on_update or [])
                    if len(waits) > max_waits:
                        extra = waits[: len(waits) - max_waits]
                        keep = waits[len(waits) - max_waits:]
                        for w in extra:
                            n_new += 1
                            out.append(mybir.InstEventSemaphore(
                                name=f"syncw-{n_new}-{inst.name}",
                                ins=[], outs=[],
                                engine=inst.engine,
                                sync_info=mybir.SyncInfo(on_wait=[w], on_update=[]),
                            ))
                        inst.sync_info = mybir.SyncInfo(on_wait=keep, on_update=ups)
                out.append(inst)
            blk.instructions = out
    return n_new


def _build_program():
    from concourse import bass, mybir, tile

    f32 = mybir.dt.float32
    bf16 = mybir.dt.bfloat16
    Alu = mybir.AluOpType
    Act = mybir.ActivationFunctionType
    DRM = mybir.MatmulPerfMode.DoubleRow

    f8 = mybir.dt.float8e4

    nc = bass.Bass("TRN2", target_bir_lowering=False, debug=False, num_devices=NCORES)

    # ---- DRAM I/O ----
    xTf = nc.dram_tensor("xTf", [BL, C, n], f32, kind="ExternalInput").ap()
    # fp8 x and x^2, packed [p, k*n+t] for DoubleRow LN1 stats
    x8p = nc.dram_tensor("x8p", [BL, 128, KT * n], f8, kind="ExternalInput").ap()
    xsq8 = nc.dram_tensor("xsq8", [BL, 128, KT * n], f8, kind="ExternalInput").ap()
    # fp8 projection weights packed for DoubleRow:
    #   wq8/wk8[p, mt*768 + kp*256 + i*128 + m] = W_eff[mt*128+m, (2kp+i)*128+p]*s
    #   wv8[p, k*768 + c] = Wv_eff[c, k*128+p]*s   (moving operand)
    wq8 = nc.dram_tensor("wq8", [128, KT * C], f8, kind="ExternalInput").ap()
    wk8 = nc.dram_tensor("wk8", [128, KT * C], f8, kind="ExternalInput").ap()
    wv8 = nc.dram_tensor("wv8", [128, KT * C], f8, kind="ExternalInput").ap()
    scq_d = nc.dram_tensor("scq", [128, 1], f32, kind="ExternalInput").ap()
    sck_d = nc.dram_tensor("sck", [128, 1], f32, kind="ExternalInput").ap()
    scv_d = nc.dram_tensor("scv", [128, 1], f32, kind="ExternalInput").ap()
    # fp8 W1 packed for DoubleRow; bf16 W2 packed per ft-tile:
    #   w1q[p, ft*768 + kp*256 + i*128 + f] = W1eff[ft*128+f, (2kp+i)*128+p] * s1
    #   w2r[p, ft*768 + mt*128 + m] = W2[mt*128+m, ft*128+p]
    w1q = nc.dram_tensor("w1q", [128, FT * C], f8, kind="ExternalInput").ap()
    w2r = nc.dram_tensor("w2r", [128, FT * C], bf16, kind="ExternalInput").ap()
    sc1_d = nc.dram_tensor("sc1", [128, 1], f32, kind="ExternalInput").ap()
    cdt = f8 if CONV_DR else bf16
    dq9 = nc.dram_tensor("dq9", [KT, 128, 9 * 128], cdt, kind="ExternalInput").ap()
    dk9 = nc.dram_tensor("dk9", [KT, 128, 9 * 128], cdt, kind="ExternalInput").ap()
    dv9 = nc.dram_tensor("dv9", [KT, 128, 9 * 128], cdt, kind="ExternalInput").ap()
    bq_d = nc.dram_tensor("bq", [C, 1], f32, kind="ExternalInput").ap()
    bk_d = nc.dram_tensor("bk", [C, 1], f32, kind="ExternalInput").ap()
    bva_d = nc.dram_tensor("bva", [C, 1], f32, kind="ExternalInput").ap()
    b1_d = nc.dram_tensor("b1", [FF, 1], f32, kind="ExternalInput").ap()
    ones_sq_d = nc.dram_tensor("ones_sq", [128, 128], bf16, kind="ExternalInput").ap()
    outT = nc.dram_tensor("outT", [BL, C, n], f32, kind="ExternalOutput").ap()

    with tile.TileContext(nc) as tc:
        with tc.tile_pool(name="P", bufs=1) as P:
            # ---- persistent SBUF (weights + per-batch activations) ----
            wq_sb = P.tile([128, KT * C], f8, name="wq8", tag="wq", bufs=1)
            wk_sb = P.tile([128, KT * C], f8, name="wk8", tag="wk", bufs=1)
            wv_sb = P.tile([128, KT * C], f8, name="wv8", tag="wv", bufs=1)
            bq6 = P.tile([128, KT], f32, name="bq6", tag="bq", bufs=1)
            bk6 = P.tile([128, KT], f32, name="bk6", tag="bk", bufs=1)
            bva6 = P.tile([128, KT], f32, name="bva6", tag="bva", bufs=1)
            b1_24 = P.tile([128, FT], f32, name="b1_24", tag="b1", bufs=1)
            ones_sq = P.tile([128, 128], bf16, name="onessq", tag="onessq", bufs=1)
            w1_sb = P.tile([128, FT * C], f8, name="w1q", tag="w1q", bufs=1)
            sc1_sb = P.tile([128, 1], f32, name="sc1", tag="sc1", bufs=1)
            scq_sb = P.tile([128, 1], f32, name="scq", tag="scq", bufs=1)
            sck_sb = P.tile([128, 1], f32, name="sck", tag="sck", bufs=1)
            scv_sb = P.tile([128, 1], f32, name="scv", tag="scv", bufs=1)

            # keep the sync queue free for batch 0's stats inputs; everything
            # needed later (biases, scales, projection weights) goes on gpsimd
            nc.gpsimd.dma_start(ones_sq[:], ones_sq_d[:, :])
            nc.gpsimd.dma_start(bq6[:], bq_d.rearrange("(t p) o -> p (t o)", p=128))
            nc.gpsimd.dma_start(bk6[:], bk_d.rearrange("(t p) o -> p (t o)", p=128))
            nc.gpsimd.dma_start(bva6[:], bva_d.rearrange("(t p) o -> p (t o)", p=128))
            nc.gpsimd.dma_start(b1_24[:], b1_d.rearrange("(t p) o -> p (t o)", p=128))
            nc.gpsimd.dma_start(sc1_sb[:], sc1_d[:, :])
            nc.gpsimd.dma_start(scq_sb[:], scq_d[:, :])
            nc.gpsimd.dma_start(sck_sb[:], sck_d[:, :])
            nc.gpsimd.dma_start(scv_sb[:], scv_d[:, :])
            nc.gpsimd.dma_start(wq_sb[:], wq8[:, :])
            nc.gpsimd.dma_start(wk_sb[:], wk8[:, :])
            nc.gpsimd.dma_start(wv_sb[:], wv8[:, :])

            pad = [P.tile([128, 34, 34], cdt, name=f"pad{k}", tag="pad", bufs=KT) for k in range(KT)]
            act8q = P.tile([128, KT * n], f8, name="a8q", tag="aq", bufs=1)
            act8k = P.tile([128, KT * M], f8, name="a8k", tag="ak", bufs=1)
            act8v = P.tile([128, KT * M], f8, name="a8v", tag="av", bufs=1)
            qT = [P.tile([128, n], bf16, name=f"qT{k}", tag="qT", bufs=KT) for k in range(KT)]
            kTt = [P.tile([128, M], bf16, name=f"kT{k}", tag="kT", bufs=KT) for k in range(KT)]
            # v tokens, one [128, 2(kv-tile), 128] block per head with the head's
            # 64 columns at its partition-half offset and zeros elsewhere, so
            # attn@v runs as accumulating DoubleRow matmuls with no tile_position
            vt8z = P.tile([128, 2, NH * 128], f8, name="vt8z", tag="vt", bufs=1)
            nc.vector.memset(vt8z[:], 0.0)
            ones8 = P.tile([128, 256], f8, name="ones8", tag="ones8", bufs=1)
            nc.vector.memset(ones8[:], 1.0)
            # half-zeroed ones stationaries: accumulate both heads' kv-sums into
            # disjoint partition halves of one PSUM tile
            ones_hf = [P.tile([128, 256], f8, name=f"oneshf{hh}", tag="oneshf", bufs=2)
                       for hh in range(2)]
            for hh in range(2):
                nc.vector.memset(ones_hf[hh][:], 0.0)
                nc.vector.memset(ones_hf[hh][:, hh * 64:hh * 64 + 64], 1.0)
                nc.vector.memset(ones_hf[hh][:, 128 + hh * 64:128 + hh * 64 + 64], 1.0)
            OT = [P.tile([128, n], bf16, name=f"OT{k}", tag="OT", bufs=KT) for k in range(KT)]
            x2b = [P.tile([128, n], bf16, name=f"x2{k}", tag="x2", bufs=KT) for k in range(KT)]
            # LN2 output: fp8, all 6 channel tiles in one buffer so DoubleRow can
            # pair adjacent k-tiles along the free dim (stride n between planes)
            xl8 = P.tile([128, KT * n], f8, name="xl8", tag="xl8", bufs=1)

            # zero the padded conv buffers once (interiors are overwritten per batch;
            # the one-element borders must stay zero)
            for k in range(KT):
                nc.vector.memset(pad[k][:], 0.0)

            # constant APs for float biases of activation ops
            czero = P.tile([128, 1], f32, name="czero", tag="cz", bufs=2)
            nc.vector.memset(czero[:], 0.0)
            nc.const_aps.aps[(f32, 0.0)] = czero[:]
            ceps = P.tile([128, 1], f32, name="ceps", tag="cz", bufs=2)
            nc.vector.memset(ceps[:], EPS)
            nc.const_aps.aps[(f32, EPS)] = ceps[:]
            cln8 = P.tile([128, 1], f32, name="cln8", tag="cln8", bufs=1)
            nc.vector.memset(cln8[:], LN_SX)
            nc.const_aps.aps[(f32, LN_SX)] = cln8[:]
            cnv = P.tile([128, 1], f32, name="cnv", tag="cnv", bufs=1)
            nc.vector.memset(cnv[:], NLN_SV)
            nc.const_aps.aps[(f32, NLN_SV)] = cnv[:]
            cl32 = P.tile([128, 1], f32, name="cl32", tag="cl32", bufs=1)
            nc.vector.memset(cl32[:], LN_32)
            nc.const_aps.aps[(f32, LN_32)] = cl32[:]

            def ln_finish(ps_mean, ps_msq, src_of, dst_write, label, bno, rstd_bias):
                mbcs, rbcs = [], []
                for ch in range(NCH):
                    mbc = P.tile([128, 512], f32, name=f"mbc{label}{bno}_{ch}", tag="mbc", bufs=2)
                    rbc = P.tile([128, 512], f32, name=f"rbc{label}{bno}_{ch}", tag="rbc", bufs=2)
                    nc.vector.tensor_scalar_mul(mbc[:], ps_mean[ch][:], 1.0 / C)
                    # rstd = 1/sqrt((msq/C) - mean^2 + eps)
                    nc.vector.tensor_mul(rbc[:], mbc[:], mbc[:])
                    nc.vector.scalar_tensor_tensor(rbc[:], ps_msq[ch][:], 1.0 / C,
                                                   rbc[:], Alu.mult, Alu.subtract)
                    # rstd = exp(-0.5*ln(var+eps)) on ACT (keeps DVE free;
                    # table accuracy ~1e-4 rel, far below bf16 noise);
                    # rstd_bias multiplies rstd by exp(rstd_bias) for free
                    nc.scalar.activation(rbc[:], rbc[:], Act.Ln, bias=EPS)
                    nc.scalar.activation(rbc[:], rbc[:], Act.Exp, scale=-0.5,
                                         bias=rstd_bias)
                    mbcs.append(mbc); rbcs.append(rbc)
                # k-major apply order so the consumer (conv k=0 / FFN kp=0)
                # unblocks after two writes instead of seven
                for k in range(KT):
                    for ch in range(NCH):
                        sl = slice(ch * 512, (ch + 1) * 512)
                        dst_write(k, ch, src_of(k, sl), mbcs[ch], rbcs[ch])

            ones8v = ones8[:].rearrange("p (i f) -> p i f", i=2)

            def ln_stats_apply(src_tiles, dst_write, label, bno, rstd_bias=0.0):
                """src_tiles: 6 bf16 [128, n] channel-major tiles.
                Stats matmuls use a ones[128,128] stationary so the channel-sums
                arrive pre-broadcast across all 128 partitions; all row math is
                then full-width DVE work and no PE broadcast is needed."""
                with tc.tile_pool(name=f"ps_ln_{label}{bno}", bufs=1, space="PSUM") as psp:
                    ps_mean = [psp.tile([128, 512], f32, name=f"psm{label}{bno}_{c}", tag="mm", bufs=4) for c in range(NCH)]
                    ps_msq = [psp.tile([128, 512], f32, name=f"psq{label}{bno}_{c}", tag="mm", bufs=4) for c in range(NCH)]
                    # squares on ACT (bf16 out), then ones-matmul stats; groups are
                    # interleaved across banks so sq tiles can double-buffer
                    for k in range(KT):
                        sqt = P.tile([128, n], bf16, name=f"sq{label}{bno}_{k}", tag="sq", bufs=2)
                        nc.scalar.activation(sqt[:], src_tiles[k][:], Act.Square)
                        for ch in range(NCH):
                            sl = slice(ch * 512, (ch + 1) * 512)
                            nc.tensor.matmul(ps_mean[ch][:], ones_sq[:], src_tiles[k][:, sl],
                                             start=(k == 0), stop=(k == KT - 1))
                            nc.tensor.matmul(ps_msq[ch][:], ones_sq[:], sqt[:, sl],
                                             start=(k == 0), stop=(k == KT - 1))
                    ln_finish(ps_mean, ps_msq, lambda k, sl: src_tiles[k][:, sl],
                              dst_write, label, bno, rstd_bias)

            def ln1_stats_apply(dst_write, bno, xb8_t, xsq8_t):
                """LN1 stats from host-precomputed fp8 x and x^2 via DoubleRow
                ones-matmuls (half the PE passes, no device squares)."""
                with tc.tile_pool(name=f"ps_ln_a{bno}", bufs=1, space="PSUM") as psp:
                    ps_mean = [psp.tile([128, 512], f32, name=f"psma{bno}_{c}", tag="mm", bufs=4) for c in range(NCH)]
                    ps_msq = [psp.tile([128, 512], f32, name=f"psqa{bno}_{c}", tag="mm", bufs=4) for c in range(NCH)]
                    for kp in range(KT // 2):
                        for ch in range(NCH):
                            sl = slice(ch * 512, (ch + 1) * 512)
                            xap = xb8_t[:, 2 * kp * n:(2 * kp + 2) * n] \
                                .rearrange("p (i t) -> p i t", i=2)[:, :, sl]
                            sap = xsq8_t[:, 2 * kp * n:(2 * kp + 2) * n] \
                                .rearrange("p (i t) -> p i t", i=2)[:, :, sl]
                            nc.tensor.matmul(ps_mean[ch][:], ones8v, xap,
                                             start=(kp == 0), stop=(kp == KT // 2 - 1),
                                             perf_mode=DRM)
                            nc.tensor.matmul(ps_msq[ch][:], ones8v, sap,
                                             start=(kp == 0), stop=(kp == KT // 2 - 1),
                                             perf_mode=DRM)
                    ln_finish(ps_mean, ps_msq, lambda k, sl: xb8_t[:, k * n:(k + 1) * n][:, sl],
                              dst_write, "a", bno, LN_SX if CONV_DR else 0.0)

            # per-batch fp8 stats inputs, double-buffered and prefetched during
            # the previous batch's attention phase
            xstats = {}

            def fetch_x(bno):
                if bno >= BL:
                    return
                t1 = P.tile([128, KT * n], f8, name=f"xb8_{bno}", tag="xb8", bufs=2)
                t2 = P.tile([128, KT * n], f8, name=f"xsq8_{bno}", tag="xsq8", bufs=2)
                nc.sync.dma_start(t1[:], x8p[bno, :, :])
                nc.sync.dma_start(t2[:], xsq8[bno, :, :])
                xstats[bno] = (t1, t2)

            def ln1_block(bno):
                """LN1 stats + normalized writes into the conv pad buffers.
                Called for batch b+1 between LN2(b) and FFN(b): the stats
                matmuls fill the PE bubble while DVE applies LN2, and the pad
                writes overlap the FFN."""
                if bno >= BL:
                    return
                xb8_t, xsq8_t = xstats.pop(bno)

                def ln1_write(k, ch, src, mbc, rbc):
                    tmp = P.tile([128, 512], f32, name=f"t1_{bno}_{k}_{ch}", tag="tmp", bufs=3)
                    nc.vector.tensor_sub(tmp[:], src, mbc[:])
                    # write normalized values into padded interior rows (fp8,
                    # prescaled by SX via the rstd bias when CONV_DR)
                    r0 = 1 + 16 * ch
                    dst = pad[k][:, r0:r0 + 16, 1:33]
                    nc.vector.tensor_mul(dst, tmp[:].rearrange("p (a c) -> p a c", a=16), rbc[:].rearrange("p (a c) -> p a c", a=16))

                ln1_stats_apply(ln1_write, bno, xb8_t, xsq8_t)

            fetch_x(0)
            ln1_block(0)
            for b in range(BL):
                # conv: fp8 DoubleRow with taps paired (4 pairs + 1 single per
                # 3x3 kernel); psum = SX*SDW*y, the elu chain emits 32*(elu+1)
                with tc.tile_pool(name=f"ps_conv{b}", bufs=1, space="PSUM") as cvp:
                    def elu_chain(ps_ap, dst_ap, width):
                        tmin = P.tile([128, width], f32, name=f"tm{b}", tag="tmpe", bufs=3)
                        et = P.tile([128, width], bf16, name=f"ee{b}", tag="ee", bufs=3)
                        nc.vector.tensor_scalar_min(tmin[:], ps_ap, 0.0)
                        if CONV_DR:
                            # 32*e^{min(y,0)} with y = psum/32
                            nc.scalar.activation(et[:], tmin[:], Act.Exp,
                                                 scale=1.0 / (SX * SDW), bias=LN_32)
                        else:
                            nc.scalar.activation(et[:], tmin[:], Act.Exp)
                        # 32*(elu+1) = relu(psum) + 32*exp(min(y,0)); the scale
                        # and the -1 are folded into the projection weights/biases
                        nc.vector.scalar_tensor_tensor(dst_ap, ps_ap, 0.0, et[:], Alu.max, Alu.add)

                    def conv_pair_ap(k, base_r, base_c, pr, rows, rstride, cstep):
                        """moving AP [128, 2, rows, 32/16]: tap pair (2pr, 2pr+1)
                        windows of the padded image (overlapping strides)."""
                        t0, t1 = 2 * pr, 2 * pr + 1
                        o0 = (t0 // 3 + base_r) * 34 + (t0 % 3) + base_c
                        o1 = (t1 // 3 + base_r) * 34 + (t1 % 3) + base_c
                        a = pad[k][:, 0:rows, 0:32:cstep].unsqueeze(1)
                        V = type(a.ap)
                        pdim = tuple(a.ap[0])
                        a.ap = V([pdim, (o1 - o0, 2), (34 * rstride, rows), (cstep, 32 // cstep)])
                        a.offset = a.offset + o0
                        return a

                    def conv_single_ap(k, base_r, base_c, tap, rows, rstride, cstep):
                        dy, dx = tap // 3, tap % 3
                        if rstride == 1:
                            return pad[k][:, base_r + dy:base_r + dy + rows, dx:dx + 32]
                        return pad[k][:, dy:dy + 32:2, dx:dx + 32:2]

                    def conv_mms(k, dt8, outs, rows, rstride, cstep):
                        """outs: list of (psum_ap, base_r); consecutive chunks
                        share each tap-pair stationary (LDW dedup friendly)."""
                        for pr in range(4):
                            wap = dt8[:, pr * 256:(pr + 1) * 256].rearrange("p (i c) -> p i c", i=2)
                            for out_ps, base_r in outs:
                                nc.tensor.matmul(out_ps, wap,
                                                 conv_pair_ap(k, base_r, 0, pr, rows, rstride, cstep),
                                                 start=(pr == 0), stop=False, perf_mode=DRM)
                        for out_ps, base_r in outs:
                            nc.tensor.matmul(out_ps, dt8[:, 1024:1152],
                                             conv_single_ap(k, base_r, 0, 8, rows, rstride, cstep),
                                             start=False, stop=True)

                    for k in range(KT):
                        dqt = P.tile([128, 9 * 128], cdt, name=f"dq{b}_{k}", tag="dq", bufs=2)
                        nc.gpsimd.dma_start(dqt[:], dq9[k, :, :])
                        pq = [cvp.tile([128, 512], f32, name=f"pcq{b}_{k}_{c}", tag="mm", bufs=4) for c in range(NCH)]
                        conv_mms(k, dqt, [(pq[ch][:], 16 * ch) for ch in range(NCH)], 16, 1, 1)
                        for ch in range(NCH):
                            elu_chain(pq[ch][:], act8q[:, k * n + ch * 512:k * n + (ch + 1) * 512], 512)
                    for k in range(KT):
                        dkt = P.tile([128, 9 * 128], cdt, name=f"dk{b}_{k}", tag="dkv", bufs=2)
                        dvt = P.tile([128, 9 * 128], cdt, name=f"dv{b}_{k}", tag="dkv", bufs=2)
                        nc.gpsimd.dma_start(dkt[:], dk9[k, :, :])
                        nc.gpsimd.dma_start(dvt[:], dv9[k, :, :])
                        pk = cvp.tile([128, M], f32, name=f"pck{b}_{k}", tag="mm", bufs=4)
                        pv = cvp.tile([128, M], f32, name=f"pcv{b}_{k}", tag="mm", bufs=4)
                        conv_mms(k, dkt, [(pk[:], 0)], 16, 2, 2)
                        conv_mms(k, dvt, [(pv[:], 0)], 16, 2, 2)
                        elu_chain(pk[:], act8k[:, k * M:(k + 1) * M], M)
                        elu_chain(pv[:], act8v[:, k * M:(k + 1) * M], M)

                    # projections -- fp8 DoubleRow over contraction-tile pairs;
                    # dequant scale + bias applied in one DVE tensor_scalar
                    def a8pair(act8, width, kp, sl2):
                        return act8[:, 2 * kp * width:(2 * kp + 2) * width] \
                            .rearrange("p (i t) -> p i t", i=2)[:, :, sl2]

                    for mt in range(KT):
                        psq = [cvp.tile([128, 512], f32, name=f"pq{b}_{mt}_{c}", tag="mm", bufs=4)
                               for c in range(NCH)]
                        for kp in range(KT // 2):
                            wap = wq_sb[:, mt * C + kp * 256:mt * C + (kp + 1) * 256] \
                                .rearrange("p (i f) -> p i f", i=2)
                            for ch in range(NCH):
                                nc.tensor.matmul(psq[ch][:], wap,
                                                 a8pair(act8q, n, kp, slice(ch * 512, (ch + 1) * 512)),
                                                 start=(kp == 0), stop=(kp == KT // 2 - 1),
                                                 perf_mode=DRM)
                        for ch in range(NCH):
                            nc.vector.tensor_scalar(qT[mt][:, ch * 512:(ch + 1) * 512], psq[ch][:],
                                                    scq_sb[:], bq6[:, mt:mt + 1],
                                                    Alu.mult, Alu.add)
                    for mt in range(KT):
                        ps = cvp.tile([128, M], f32, name=f"pk{b}_{mt}", tag="mm", bufs=4)
                        for kp in range(KT // 2):
                            wap = wk_sb[:, mt * C + kp * 256:mt * C + (kp + 1) * 256] \
                                .rearrange("p (i f) -> p i f", i=2)
                            nc.tensor.matmul(ps[:], wap, a8pair(act8k, M, kp, slice(0, M)),
                                             start=(kp == 0), stop=(kp == KT // 2 - 1),
                                             perf_mode=DRM)
                        nc.vector.tensor_scalar(kTt[mt][:, :], ps[:],
                                                sck_sb[:], bk6[:, mt:mt + 1],
                                                Alu.mult, Alu.add)
                    for mt2 in range(2):
                        psv = [cvp.tile([128, w], f32, name=f"pv{b}_{mt2}_{c}", tag="mm", bufs=4)
                               for c, w in [(0, 512), (1, 256)]]
                        for kp in range(KT // 2):
                            aap = a8pair(act8v, M, kp, slice(mt2 * 128, (mt2 + 1) * 128))
                            for ch, w in [(0, 512), (1, 256)]:
                                nc.tensor.matmul(psv[ch][:], aap,
                                                 wv_sb[:, 2 * kp * C:(2 * kp + 2) * C]
                                                 .rearrange("p (i c) -> p i c", i=2)[:, :, ch * 512:ch * 512 + w],
                                                 start=(kp == 0), stop=(kp == KT // 2 - 1),
                                                 perf_mode=DRM)
                        vtv = vt8z[:].rearrange("p m (h q d) -> p m h q d", h=NH, q=2)
                        for ch, w in [(0, 512), (1, 256)]:
                            # v tokens in fp8, prescaled by SV/s_wv (folded out
                            # via sinv); even/odd heads land in their q-halves
                            g0, nh = ch * 8, w // 64
                            src = psv[ch][:].rearrange("p (h d) -> p h d", d=64)
                            for par in range(2):
                                nc.vector.tensor_scalar(
                                    vtv[:, mt2:mt2 + 1, g0 + par:g0 + nh:2, par:par + 1, :],
                                    src[:, par:nh:2, :],
                                    scv_sb[:], None, Alu.mult)

                if b == 0:
                    # one-time fp8 W1 load; queued here so batch 0's conv
                    # weights (same gpsimd queue) aren't delayed behind it
                    for half in range(4):
                        slh = slice(half * (FT * C // 4), (half + 1) * (FT * C // 4))
                        nc.gpsimd.dma_start(w1_sb[:, slh], w1q[:, slh])

                # prefetch next batch's stats inputs while the sync queue is idle
                fetch_x(b + 1)

                # ---------------- attention ----------------
                # software-pipelined over head pairs: scores(j+1) are emitted
                # before sum/AV(j) so the PE streams while ACT runs the exps
                with tc.tile_pool(name=f"ps_at{b}", bufs=1, space="PSUM") as atp:
                    def att_scores(j):
                        # exp(scores) in fp8, kv tiles stacked for DoubleRow;
                        # alternate the two heads' row-halves so the PE streams
                        # both halves concurrently
                        ET2 = [P.tile([128, 2, n], f8, name=f"ET{b}_{j}_{hh}", tag="ET", bufs=4)
                               for hh in range(2)]
                        for mt in range(2):
                            for ch in range(NCH):
                                for hh in range(2):
                                    bp = 64 * hh
                                    ps = atp.tile([128, 512], f32, name=f"pss{b}_{j}_{hh}_{mt}_{ch}", tag="mm", bufs=4)
                                    nc.tensor.matmul(ps[:],
                                                     kTt[j][bp:bp + 64, mt * 128:(mt + 1) * 128],
                                                     qT[j][bp:bp + 64, ch * 512:(ch + 1) * 512],
                                                     tile_position=(bp, 0))
                                    nc.scalar.activation(ET2[hh][:, mt:mt + 1, ch * 512:(ch + 1) * 512],
                                                         ps[:], Act.Exp, scale=0.125)
                        return ET2

                    def att_finish(j, ET2):
                        # kv-sums of both heads accumulate into disjoint partition
                        # halves of one PSUM tile (half-zeroed ones stationaries),
                        # so Ln/Exp run once per chunk at full width
                        sinv = [P.tile([128, 512], f32, name=f"si{b}_{j}_{c}", tag="sinv", bufs=4)
                                for c in range(NCH)]
                        for ch in range(NCH):
                            sum_ps = atp.tile([128, 512], f32, name=f"psum{b}_{j}_{ch}", tag="bc", bufs=2)
                            for hh in range(2):
                                nc.tensor.matmul(sum_ps[:],
                                                 ones_hf[hh][:].rearrange("p (i f) -> p i f", i=2),
                                                 ET2[hh][:, :, ch * 512:(ch + 1) * 512],
                                                 start=(hh == 0), stop=(hh == 1),
                                                 perf_mode=DRM)
                            # 1/(s*SV) = exp(-ln(s) - ln SV) on ACT
                            nc.scalar.activation(sinv[ch][:], sum_ps[:], Act.Ln)
                            nc.scalar.activation(sinv[ch][:], sinv[ch][:], Act.Exp,
                                                 scale=-1.0, bias=NLN_SV)
                        po = [atp.tile([128, 512], f32, name=f"po{b}_{j}_{c}", tag="o", bufs=2)
                              for c in range(NCH)]
                        vtr = vt8z[:]
                        for ch in range(NCH):
                            for hh in range(2):
                                h = 2 * j + hh
                                nc.tensor.matmul(po[ch][:],
                                                 vtr[:, :, h * 128:(h + 1) * 128],
                                                 ET2[hh][:, :, ch * 512:(ch + 1) * 512],
                                                 start=(hh == 0), stop=(hh == 1),
                                                 perf_mode=DRM)
                        for ch in range(NCH):
                            sl = slice(ch * 512, (ch + 1) * 512)
                            nc.vector.tensor_mul(OT[j][:, sl], po[ch][:], sinv[ch][:])

                    ET_prev = att_scores(0)
                    for j in range(1, NH // 2):
                        ET_cur = att_scores(j)
                        att_finish(j - 1, ET_prev)
                        ET_prev = ET_cur
                    att_finish(NH // 2 - 1, ET_prev)

                # ---------------- residual + LN2 ----------------
                for k in range(KT):
                    for ch in range(NCH):
                        sl = slice(ch * 512, (ch + 1) * 512)
                        xf = P.tile([128, 512], f32, name=f"xf{b}_{k}_{ch}", tag="xf", bufs=3)
                        nc.sync.dma_start(xf[:], xTf[b, k * 128:(k + 1) * 128, sl])
                        nc.vector.scalar_tensor_tensor(x2b[k][:, sl], OT[k][:, sl], bva6[:, k:k + 1], xf[:],
                                                       Alu.add, Alu.add)

                def ln2_write(k, ch, src, mbc, rbc):
                    tmp = P.tile([128, 512], f32, name=f"t2_{b}_{k}_{ch}", tag="tmp", bufs=3)
                    nc.vector.tensor_sub(tmp[:], src, mbc[:])
                    # rbc carries exp(ln 8) = SX, so this writes xn*8 in fp8e4
                    nc.vector.tensor_mul(xl8[:, k * n + ch * 512:k * n + (ch + 1) * 512],
                                         tmp[:], rbc[:])

                ln_stats_apply(x2b, ln2_write, "c", b, rstd_bias=LN_SX)

                # next batch's LN1: stats fill the PE bubble while DVE drains
                # the LN2 apply; pad writes run under the FFN
                ln1_block(b + 1)

                # ---------------- FFN (fp8 DoubleRow h1, bf16 h2) + residual ----------------
                # software-pipelined: h1(ft+1) is emitted before h2(ft) so the
                # PE streams through the gelu latency
                with tc.tile_pool(name=f"ps_ffn{b}", bufs=1, space="PSUM") as ffp:
                    for ch in range(NCH):
                        sl = slice(ch * 512, (ch + 1) * 512)
                        ph2 = [ffp.tile([128, 512], f32, name=f"ph2_{b}_{ch}_{mt}", tag="h2", bufs=6)
                               for mt in range(KT)]
                        ph1s, w2bs = {}, {}

                        def fetch_w2(ft):
                            if ft >= FT:
                                return
                            w2b = P.tile([128, C], bf16, name=f"w2_{b}_{ch}_{ft}", tag="w2", bufs=4)
                            nc.gpsimd.dma_start(w2b[:], w2r[:, ft * C:(ft + 1) * C])
                            w2bs[ft] = w2b

                        def emit_h1(ft):
                            ph1 = ffp.tile([128, 512], f32, name=f"ph1_{b}_{ch}_{ft}", tag="h1", bufs=2)
                            for kp in range(KT // 2):
                                w1ap = w1_sb[:, ft * C + kp * 256: ft * C + (kp + 1) * 256] \
                                    .rearrange("p (i f) -> p i f", i=2)
                                xap = xl8[:, 2 * kp * n:(2 * kp + 2) * n] \
                                    .rearrange("p (i t) -> p i t", i=2)[:, :, sl]
                                nc.tensor.matmul(ph1[:], w1ap, xap,
                                                 start=(kp == 0), stop=(kp == KT // 2 - 1),
                                                 perf_mode=DRM)
                            ph1s[ft] = ph1

                        fetch_w2(0)
                        fetch_w2(1)
                        emit_h1(0)
                        for ft in range(FT):
                            fetch_w2(ft + 2)
                            if ft + 1 < FT:
                                emit_h1(ft + 1)
                            gt = P.tile([128, 512], bf16, name=f"g_{b}_{ch}_{ft}", tag="g", bufs=3)
                            nc.scalar.activation(gt[:], ph1s.pop(ft)[:], Act.Gelu,
                                                 bias=b1_24[:, ft:ft + 1], scale=sc1_sb[:])
                            w2b = w2bs.pop(ft)
                            for mt in range(KT):
                                nc.tensor.matmul(ph2[mt][:],
                                                 w2b[:, mt * 128:(mt + 1) * 128],
                                                 gt[:],
                                                 start=(ft == 0), stop=(ft == FT - 1))
                        for mt in range(KT):
                            xf2 = P.tile([128, 512], f32, name=f"xf2_{b}_{ch}_{mt}", tag="xf", bufs=3)
                            nc.sync.dma_start(xf2[:], xTf[b, mt * 128:(mt + 1) * 128, sl])
                            ub = P.tile([128, 512], f32, name=f"u_{b}_{ch}_{mt}", tag="tmp", bufs=3)
                            nc.vector.scalar_tensor_tensor(ub[:], OT[mt][:, sl], bva6[:, mt:mt + 1], xf2[:],
                                                           Alu.add, Alu.add)
                            ob = P.tile([128, 512], f32, name=f"o_{b}_{ch}_{mt}", tag="ob", bufs=3)
                            nc.vector.tensor_add(ob[:], ub[:], ph2[mt][:])
                            # stores wait on compute; keep them off the weight
                            # queues so they can't head-of-line block prefetches
                            nc.scalar.dma_start(outT[b, mt * 128:(mt + 1) * 128, sl], ob[:])
    n_hoisted = _split_sync_waits(nc)
    print(f"_split_sync_waits: hoisted waits onto {n_hoisted} carrier instructions")
    return nc


def _host_prep(inputs):
    """Fold LN/BN affines into weights; build packed bf16 arrays."""
    f = lambda k: np.asarray(inputs[k], np.float32)
    bfc = lambda a: np.ascontiguousarray(a.astype(ml_dtypes.bfloat16))
    x = f("x")                         # (B, n, C)
    ln1_g, ln1_b = f("ln1_g"), f("ln1_b")
    ln2_g, ln2_b = f("ln2_g"), f("ln2_b")

    f8c = lambda a: np.clip(a, -240.0, 240.0).astype(ml_dtypes.float8_e4m3)
    prep = {}
    xT = np.ascontiguousarray(x.transpose(0, 2, 1))   # (B, C, n)
    prep["xTf"] = xT
    # fp8 x and x^2 packed [b, p, k*n + t] for DoubleRow LN1 stats
    xp = xT.reshape(B, KT, 128, Ht * Wt).transpose(0, 2, 1, 3).reshape(B, 128, KT * Ht * Wt)
    prep["x8p"] = np.ascontiguousarray(f8c(xp))
    prep["xsq8"] = np.ascontiguousarray(f8c(xp * xp))

    diag9 = {}
    badj = {}
    for nm in ["q", "k", "v"]:
        w = f(f"dw_w_{nm}")[:, 0]                     # (C,3,3)
        w_eff = w * ln1_g[:, None, None]
        cb = f(f"dw_b_{nm}") + ln1_b * w.sum((1, 2))  # exact only if ln1_b == 0 (boundary)
        assert np.abs(cb).max() < 1e-30, "nonzero conv bias not implemented on device"
        sc = f(f"bn_g_{nm}") / np.sqrt(f(f"bn_v_{nm}") + EPS)
        sh = f(f"bn_b_{nm}") - f(f"bn_m_{nm}") * sc
        W = f(f"W_{nm}")
        W_eff = W * sc[None, :]
        # with CONV_DR the device act is 32*(elu+1); fold the /32 into W here
        CA = SX * SDW if CONV_DR else 1.0
        s_w = 2.0 ** np.floor(np.log2(224.0 * CA / max(np.abs(W_eff).max(), 1e-30)))
        Wq8 = f8c(W_eff * (s_w / CA))
        # the device multiplies with the fp8 weights, so the elu+1 "-1" fold
        # must subtract the row sums of the QUANTIZED weights or a constant
        # per-channel offset (Wq-W).sum(1) leaks into the output
        W_deq = Wq8.astype(np.float32) * (CA / s_w)
        b_eff = f(f"b_{nm}") + W @ sh - W_deq.sum(1)
        # pack tap matrices: 4 DoubleRow pairs + 1 single when CONV_DR
        # (diag pairs [pr, i, c]), else 9 diagonal taps
        d = np.zeros((KT, 128, 9 * 128), np.float32)
        wpack = w_eff * SDW if CONV_DR else w_eff
        for kt in range(KT):
            ww = wpack[kt * 128:(kt + 1) * 128]       # (128,3,3)
            for tap in range(9):
                dy, dx = tap // 3, tap % 3
                d[kt, np.arange(128), tap * 128 + np.arange(128)] = ww[:, dy, dx]
        diag9[nm] = f8c(d) if CONV_DR else bfc(d)
        badj[nm] = b_eff
        if nm == "v":
            # moving operand: wv8[p, k*768 + c] = (W_eff/CA).T[k*128+p, c] * s
            wv = Wq8.T.reshape(KT, 128, C).transpose(1, 0, 2).reshape(128, KT * C)
            prep["wv8"] = np.ascontiguousarray(wv)
            prep["scv"] = np.full((128, 1), SV / s_w, np.float32)
        else:
            # stationary: w8[p, mt*768 + kp*256 + i*128 + m] = Wq8[mt*128+m, (2kp+i)*128+p]
            wq = Wq8.reshape(KT, 128, KT, 128).transpose(3, 0, 2, 1).reshape(128, KT * C)
            prep[f"w{nm}8"] = np.ascontiguousarray(wq)
            prep[f"sc{nm}"] = np.full((128, 1), 1.0 / s_w, np.float32)
    prep["dq9"], prep["dk9"], prep["dv9"] = diag9["q"], diag9["k"], diag9["v"]
    prep["bq"] = badj["q"].reshape(C, 1)
    prep["bk"] = badj["k"].reshape(C, 1)
    prep["bva"] = badj["v"].reshape(C, 1)

    W1 = f("W1") * ln2_g[None, :]                     # (FF, C)
    b1 = f("b1") + f("W1") @ ln2_b
    W2 = f("W2")                                      # (C, FF)
    assert np.abs(f("b2")).max() < 1e-30, "nonzero b2 not implemented on device"
    # fp8e4 (TRN: max +-240) DoubleRow packing, power-of-2 per-tensor scale
    s1 = 2.0 ** np.floor(np.log2(224.0 / max(np.abs(W1).max(), 1e-30)))
    # w1q[p, ft*768 + kp*256 + i*128 + f] = W1[ft*128+f, (2kp+i)*128+p] * s1
    w1q = (W1 * s1).reshape(FT, 128, KT, 128).transpose(3, 0, 2, 1).reshape(128, FT * C)
    # w2r[p, ft*768 + mt*128 + m] = W2[mt*128+m, ft*128+p]
    w2r = W2.T.reshape(FT, 128, C).transpose(1, 0, 2).reshape(128, FT * C)
    prep["w1q"] = np.ascontiguousarray(f8c(w1q))
    prep["w2r"] = bfc(w2r)
    prep["sc1"] = np.full((128, 1), 1.0 / (s1 * SX), np.float32)
    prep["b1"] = b1.reshape(FF, 1)
    prep["ones_sq"] = np.ones((128, 128), ml_dtypes.bfloat16)
    return prep


def kernel(**inputs):
    from concourse.bass_utils import run_bass_kernel_spmd

    _patch_compiler(ldw_opt=_BUILD_CACHE.get("ldw_opt", False))
    if "nc" not in _BUILD_CACHE:
        _BUILD_CACHE["nc"] = _build_program()
    nc = _BUILD_CACHE["nc"]

    prep = _host_prep(inputs)
    SHARDED = ("xTf", "x8p", "xsq8")
    shared = {k: v for k, v in prep.items() if k not in SHARDED}
    in_maps = []
    for c in range(NCORES):
        im = dict(shared)
        for k in SHARDED:
            im[k] = np.ascontiguousarray(prep[k][c * BL:(c + 1) * BL])
        in_maps.append(im)

    res = run_bass_kernel_spmd(nc, in_maps, list(range(NCORES)),
                               **_BUILD_CACHE.get("run_kwargs", {}))
    _BUILD_CACHE["last_results"] = res
    outs = [res.results[c]["outT"].transpose(0, 2, 1) for c in range(NCORES)]
    return np.ascontiguousarray(np.concatenate(outs, 0).astype(np.float32))

